# revision 13
# baseline (speedup 1.0000x reference)
"""GQA attention (B=2, T=2048, DIM=2048, NH=32, NKV=8, HD=64) with RoPE, causal,
on 8 TRN2 NeuronCores.

Sharding: data-parallel over B (2) x tensor-parallel over kv-head groups (4).
Core i handles batch i//4 and kv heads {2g, 2g+1} (g = i%4), i.e. q heads
8g..8g+8. wq/wk/wv column-parallel, wo row-parallel; host sums the 4 partial
outputs per batch.

Everything on-device is feature-major ("transposed"): x^T, Q^T, K^T are
[feature, t] so no on-device transposes are needed anywhere:
  QT[d,t] = wq^T x^T;  scoresT[s,q] = (KT slice)^T @ QT;  OT[d,q] = V^T @ PT;
  outT[o,t] = wo^T @ OT.  Host transposes the final [o,t] back to [t,o].

RoPE de-interleave: within each head the 64 features are permuted to
[32 real | 32 imag] (host permutes wq/wk columns), so rot = q*C + swap(q)*S
with the half-swap done by SBUF->SBUF DMA.

Attention inner loop (the key perf structure): for each 128-wide s-block, ONE
PSUM tile sc[128,1024] holds BOTH kv-head halves: cols 0:512 = scores of head
(slab,h0) against KT rows 0:64, cols 512:1024 = head (slab,h1) against KT rows
64:128.  The two K=64 score matmuls land on disjoint PE row groups (0,0) /
(64,0) and are emitted back-to-back with attention MMs at the highest
scheduling priority, so the hardware co-streams them (2x throughput for the
64-deep contraction).  One exp ACT instruction covers both halves; causal
masking multiplies a host-precomputed 0/1 tile only on the 4 diagonal s-blocks
of each q-chunk.  PV runs as two K=128 matmuls (one per half) accumulating
into per-half [65,512] PSUM tiles whose appended ones-column yields the
softmax denominator for free; 1/denom via reciprocal_approx_fast + gpsimd
partition_broadcast.

PSUM budget (8 banks): sc 2 bufs x 2 banks + ot_h0/ot_h1 1 buf x 1 bank each
+ proj/wo accumulator 2 bufs x 1 bank = 8.

Emission order per t-block: attention(tb) FIRST (highest priority, keeps the
co-stream pairs adjacent and PV launching right after exp), then proj(tb+1),
then wo(tb) as gap fillers for the ACT-paced exp stream.  All x tiles are
DMA-prefetched at kernel start across the sync/vector/scalar queues so the PE
never waits on input DMA mid-kernel.

All matmul operands are fp16 (KERNEL_MM_DTYPE also allows bf16/f32r); the PE
streams 2-byte operands at full 2.4 GHz rate, fp32 PSUM accumulate.
"""

import numpy as np

B, T, DIM = 2, 2048, 2048
NH, NKV, HD = 32, 8, 64
G = 4            # tensor-parallel groups
QH = NH // G     # 8 local q heads
SLABS = 4
KTILES = DIM // 128
TBLK = T // 512

_CACHE = {}
import os as _os
_MM_DTYPE = _os.environ.get("KERNEL_MM_DTYPE", "fp16")


def _to_mm_dtype(x: np.ndarray) -> np.ndarray:
    if _MM_DTYPE == "bf16":
        import ml_dtypes
        return np.ascontiguousarray(x, dtype=np.float32).astype(ml_dtypes.bfloat16)
    if _MM_DTYPE == "fp16":
        return np.ascontiguousarray(x, dtype=np.float32).astype(np.float16)
    return _round_f32r(x)


def _round_f32r(x: np.ndarray) -> np.ndarray:
    """Round f32 to the float32r grid (11 mantissa bits, round-to-nearest-even)."""
    x = np.ascontiguousarray(x, dtype=np.float32)
    xi = x.view(np.uint32).copy()
    shift = 12  # keep 11 mantissa bits
    lsb = (xi >> shift) & 1
    xi = (xi + ((1 << (shift - 1)) - 1) + lsb) & np.uint32(~((1 << shift) - 1) & 0xFFFFFFFF)
    return xi.view(np.float32)


def _build():
    import concourse.bass as bass
    import concourse.mybir as mybir
    import concourse.tile as tile
    from concourse import bacc

    F32 = mybir.dt.float32
    F32R = {"bf16": mybir.dt.bfloat16, "fp16": mybir.dt.float16,
            "f32r": mybir.dt.float32r}[_MM_DTYPE]
    EXP = mybir.ActivationFunctionType.Exp

    nc = bacc.Bacc("TRN2", target_bir_lowering=False, debug=False, num_devices=8)

    xT = nc.dram_tensor("xT", [DIM, T], F32R, kind="ExternalInput").ap()
    wq = nc.dram_tensor("wq", [DIM, QH * HD], F32R, kind="ExternalInput").ap()
    wk = nc.dram_tensor("wk", [DIM, 2 * HD], F32R, kind="ExternalInput").ap()
    wv = nc.dram_tensor("wv", [DIM, 2 * HD], F32R, kind="ExternalInput").ap()
    wo = nc.dram_tensor("wo", [QH * HD, DIM], F32R, kind="ExternalInput").ap()
    c4 = nc.dram_tensor("c4", [128, T], F32R, kind="ExternalInput").ap()
    s4 = nc.dram_tensor("s4", [128, T], F32R, kind="ExternalInput").ap()
    vones = nc.dram_tensor("vones", [128, 16 * 65], F32R, kind="ExternalInput").ap()
    msk = nc.dram_tensor("msk", [128, 4096], F32R, kind="ExternalInput").ap()
    outT = nc.dram_tensor("outT", [DIM, T], F32R, kind="ExternalOutput").ap()

    from contextlib import ExitStack

    with tile.TileContext(nc) as tc, ExitStack() as ctx:
        # ---------- persistent tiles ----------
        pers = ctx.enter_context(tc.tile_pool(name="pers", bufs=1))
        KT = pers.tile([128, T], F32R, tag="kt", name="kt")
        V0 = pers.tile([128, 16 * 65], F32R, tag="v0", name="v0")
        V1 = pers.tile([128, 16 * 65], F32R, tag="v1", name="v1")
        MSK = pers.tile([128, 4096], F32R, tag="msk", name="msk_sb")
        C4 = pers.tile([128, T], F32R, tag="c4", name="c4_sb")
        S4 = pers.tile([128, T], F32R, tag="s4", name="s4_sb")
        WQ = pers.tile([128, KTILES * 512], F32R, tag="wq", name="wq_sb")
        WK = pers.tile([128, KTILES * 128], F32R, tag="wk", name="wk_sb")
        WV = pers.tile([128, KTILES * 128], F32R, tag="wv", name="wv_sb")
        WO = [pers.tile([128, T], F32R, tag=f"wo{s}", name=f"wo{s}") for s in range(SLABS)]
        # all x tiles resident for the whole kernel: [tb][k] -> [128, 512]
        XT = [[pers.tile([128, 512], F32R, tag=f"x{tb}_{k}", name=f"x{tb}_{k}")
               for k in range(KTILES)] for tb in range(TBLK)]

        rot = ctx.enter_context(tc.tile_pool(name="rot", bufs=2))
        work = ctx.enter_context(tc.tile_pool(name="work", bufs=2))
        ptp = ctx.enter_context(tc.tile_pool(name="ptp", bufs=4))
        misc = ctx.enter_context(tc.tile_pool(name="misc", bufs=2))
        osbp = ctx.enter_context(tc.tile_pool(name="osbp", bufs=6))
        ps_acc = ctx.enter_context(tc.tile_pool(name="ps_acc", bufs=2, space="PSUM"))
        ps_sc = ctx.enter_context(tc.tile_pool(name="ps_sc", bufs=2, space="PSUM"))
        ps_ot = ctx.enter_context(tc.tile_pool(name="ps_ot", bufs=1, space="PSUM"))

        # ---------- input DMA: everything prefetched up front ----------
        # The gpsimd queue is reserved for the latency-critical rope swap
        # DMAs (tiny SBUF->SBUF transfers emitted inside emit_proj) -- bulk
        # loads queued ahead of them starve the rope chain and stall the PE.
        # sync: x0-even, x1, wo (+ outT stores later).
        # scalar: x0-odd, wq, rope tables (tb0 first), masks, wk, wv, x2, x3.
        for k in range(0, KTILES, 2):
            nc.sync.dma_start(XT[0][k][:], xT[k * 128:(k + 1) * 128, 0:512])
        for k in range(1, KTILES, 2):
            nc.scalar.dma_start(XT[0][k][:], xT[k * 128:(k + 1) * 128, 0:512])
        for k in range(KTILES):
            nc.sync.dma_start(XT[1][k][:], xT[k * 128:(k + 1) * 128, 512:1024])
        for s in range(SLABS):
            nc.sync.dma_start(WO[s][:], wo[s * 128:(s + 1) * 128, :])
        for k in range(KTILES):
            nc.scalar.dma_start(WQ[:, k * 512:(k + 1) * 512], wq[k * 128:(k + 1) * 128, :])
        nc.scalar.dma_start(C4[:, 0:512], c4[:, 0:512])
        nc.scalar.dma_start(S4[:, 0:512], s4[:, 0:512])
        nc.scalar.dma_start(MSK[:], msk[:])
        for k in range(KTILES):
            nc.scalar.dma_start(WK[:, k * 128:(k + 1) * 128], wk[k * 128:(k + 1) * 128, :])
        for k in range(KTILES):
            nc.scalar.dma_start(WV[:, k * 128:(k + 1) * 128], wv[k * 128:(k + 1) * 128, :])
        nc.gpsimd.dma_start(V0[:], vones[:])
        nc.gpsimd.dma_start(V1[:], vones[:])
        nc.scalar.dma_start(C4[:, 512:2048], c4[:, 512:2048])
        nc.scalar.dma_start(S4[:, 512:2048], s4[:, 512:2048])
        for k in range(KTILES):
            nc.scalar.dma_start(XT[2][k][:], xT[k * 128:(k + 1) * 128, 1024:1536])
        for k in range(KTILES):
            nc.scalar.dma_start(XT[3][k][:], xT[k * 128:(k + 1) * 128, 1536:2048])

        QTr_by_tb = {}

        def emit_proj(tb):
            t_sl = slice(tb * 512, (tb + 1) * 512)
            xts = XT[tb]
            QTr = [None] * SLABS

            def qk_group(s):
                # s < SLABS: q slab s; s == SLABS: k
                ps = ps_acc.tile([128, 512], F32, tag="acc", name="pq")
                for k in range(KTILES):
                    if s < SLABS:
                        lhs = WQ[:, k * 512 + s * 128: k * 512 + (s + 1) * 128]
                    else:
                        lhs = WK[:, k * 128:(k + 1) * 128]
                    nc.tensor.matmul(ps[:], lhs, xts[k][:],
                                     start=(k == 0), stop=(k == KTILES - 1))
                if s < SLABS:
                    dst_t = rot.tile([128, 512], F32R, tag=f"qtr{s}", name=f"qtr{s}")
                    QTr[s] = dst_t
                    dst = dst_t[:]
                else:
                    dst = KT[:, t_sl]
                # single PSUM read (frees the accumulator slot); rest of the
                # rope math is fp16 SBUF-only for the faster DVE tiers
                q_sb = work.tile([128, 512], F32R, tag="qsb", name="qsb")
                nc.vector.tensor_copy(q_sb[:], ps[:])
                q_sw = work.tile([128, 512], F32R, tag="qsw", name="qsw")
                for o in (0, 64):
                    nc.gpsimd.dma_start(q_sw[o:o + 32, :], q_sb[o + 32:o + 64, :])
                    nc.gpsimd.dma_start(q_sw[o + 32:o + 64, :], q_sb[o:o + 32, :])
                m1 = work.tile([128, 512], F32R, tag="m1", name="m1")
                nc.vector.tensor_mul(m1[:], q_sb[:], C4[:, t_sl])
                m2 = work.tile([128, 512], F32R, tag="m2", name="m2")
                nc.vector.tensor_mul(m2[:], q_sw[:], S4[:, t_sl])
                nc.vector.tensor_add(dst, m1[:], m2[:])

            def v_group(i):
                sbi = tb * 4 + i
                pv = ps_acc.tile([128, 128], F32, tag="acc", name="pv",
                                 padded_shape=[128, 512])
                for k in range(KTILES):
                    nc.tensor.matmul(pv[:], xts[k][:, i * 128:(i + 1) * 128],
                                     WV[:, k * 128:(k + 1) * 128],
                                     start=(k == 0), stop=(k == KTILES - 1))
                nc.vector.tensor_copy(V0[:, sbi * 65: sbi * 65 + 64], pv[:, 0:64])
                nc.vector.tensor_copy(V1[:, sbi * 65: sbi * 65 + 64], pv[:, 64:128])

            if tb == 0:
                # attention(0) needs qtr0 + kt + v blocks first; q2/q3 last
                for g in (0, 1, SLABS):
                    qk_group(g)
                for i in range(4):
                    v_group(i)
                qk_group(2)
                qk_group(3)
            else:
                # attention(tb) starts from s-block 0: qtr first, own k/v last
                for g in (0, 1, 2, 3, SLABS):
                    qk_group(g)
                for i in range(4):
                    v_group(i)
            QTr_by_tb[tb] = QTr

        def emit_attention(qc):
            QTr = QTr_by_tb.pop(qc)
            nblk = (qc + 1) * 4
            OTNr = []
            for s in range(SLABS):
                ot0 = ps_ot.tile([65, 512], F32, tag="ot0", name="ot0")
                ot1 = ps_ot.tile([65, 512], F32, tag="ot1", name="ot1")
                # PV matmuls are emitted one s-block BEHIND the score pair so
                # their scheduler priority is lower than the next score pair:
                # the co-streamed (row-group 0/64) score MMs stay adjacent in
                # the PE queue even when an exp completes mid-pair.
                pend = None

                def flush_pv(sb_):
                    nc.tensor.matmul(ot0[:], V0[:, sb_ * 65: sb_ * 65 + 65],
                                     pend[:, 0:512],
                                     start=(sb_ == 0), stop=(sb_ == nblk - 1))
                    nc.tensor.matmul(ot1[:], V1[:, sb_ * 65: sb_ * 65 + 65],
                                     pend[:, 512:1024],
                                     start=(sb_ == 0), stop=(sb_ == nblk - 1))

                for sb in range(nblk):
                    sc = ps_sc.tile([128, 1024], F32, tag="sc", name="sc")
                    nc.tensor.matmul(sc[:, 0:512],
                                     KT[0:64, sb * 128:(sb + 1) * 128],
                                     QTr[s][0:64, :], start=True, stop=True)
                    nc.tensor.matmul(sc[:, 512:1024],
                                     KT[64:128, sb * 128:(sb + 1) * 128],
                                     QTr[s][64:128, :], start=True, stop=True)
                    if sb > 0:
                        flush_pv(sb - 1)
                    pt = ptp.tile([128, 1024], F32R, tag="pt", name="pt")
                    nc.scalar.activation(pt[:], sc[:], EXP)
                    off = sb * 128 - qc * 512
                    if off >= 0:  # diagonal s-block: apply causal 0/1 mask
                        v = off // 128
                        nc.vector.tensor_mul(pt[:], pt[:],
                                             MSK[:, v * 1024:(v + 1) * 1024])
                    pend = pt
                flush_pv(nblk - 1)
                otn = rot.tile([128, 512], F32R, tag=f"otnr{s}", name=f"otnr{s}")
                OTNr.append(otn)
                bcs = []
                for h, ot in ((0, ot0), (1, ot1)):
                    dsb = misc.tile([1, 512], F32, tag=f"dsb{h}", name=f"dsb{h}")
                    nc.vector.tensor_copy(dsb[:], ot[64:65, :])
                    rcf = misc.tile([1, 512], F32, tag=f"rcf{h}", name=f"rcf{h}")
                    nc.vector.reciprocal_approx_fast(rcf[:], dsb[:])
                    bc = misc.tile([64, 512], F32, tag=f"bc{h}", name=f"bc{h}")
                    nc.gpsimd.partition_broadcast(bc[:], rcf[:])
                    bcs.append(bc)
                nc.vector.tensor_mul(otn[0:64, :], ot0[0:64, :], bcs[0][:])
                nc.vector.tensor_mul(otn[64:128, :], ot1[0:64, :], bcs[1][:])
            return OTNr

        def emit_wo(tb, OTNr):
            t_sl = slice(tb * 512, (tb + 1) * 512)
            for ob in range(16):
                po = ps_acc.tile([128, 512], F32, tag="acc", name="po")
                for s in range(SLABS):
                    nc.tensor.matmul(po[:], WO[s][:, ob * 128:(ob + 1) * 128],
                                     OTNr[s][:], start=(s == 0), stop=(s == SLABS - 1))
                osb = osbp.tile([128, 512], F32R, tag="osb", name="osb")
                nc.vector.tensor_copy(osb[:], po[:])
                nc.sync.dma_start(outT[ob * 128:(ob + 1) * 128, t_sl], osb[:])

        emit_proj(0)
        for tb in range(TBLK):
            OTNr = emit_attention(tb)
            if tb + 1 < TBLK:
                emit_proj(tb + 1)
            emit_wo(tb, OTNr)

    nc.compile()
    return nc


def _prep_inputs(x, freqs_cos, freqs_sin, wq, wk, wv, wo):
    """Build the 8 per-core input maps (host-side sharding + layout prep)."""
    x = np.asarray(x, dtype=np.float32)
    freqs_cos = np.asarray(freqs_cos, dtype=np.float32)
    freqs_sin = np.asarray(freqs_sin, dtype=np.float32)
    wq = np.asarray(wq, dtype=np.float32)
    wk = np.asarray(wk, dtype=np.float32)
    wv = np.asarray(wv, dtype=np.float32)
    wo = np.asarray(wo, dtype=np.float32)

    # de-interleave permutation within a head: [2j] then [2j+1]
    deint = np.concatenate([np.arange(0, HD, 2), np.arange(1, HD, 2)])

    # rope tables [128, T]: row r uses freq index r % 32; sign of sin flips
    # per 32-block (real-out blocks get -sin)
    cosT = freqs_cos.T  # [32, T]
    sinT = freqs_sin.T
    c4 = np.tile(cosT, (4, 1)).astype(np.float32)
    s4 = np.concatenate([-sinT, sinT, -sinT, sinT], axis=0).astype(np.float32)

    vones = np.zeros((128, 16 * 65), dtype=np.float32)
    vones[:, 64::65] = 1.0
    # diagonal masks: variant v is for the s-block at offset off=128*v inside
    # the 512-wide q window; both halves share it: msk[p, v*1024 + h*512 + q]
    # = 1 if (128v + p) <= q else 0
    msk = np.zeros((128, 4096), dtype=np.float32)
    p_ = np.arange(128)[:, None]
    q_ = np.arange(512)[None, :]
    for v in range(4):
        blk = (128 * v + p_) <= q_
        for h in range(2):
            msk[:, v * 1024 + h * 512: v * 1024 + (h + 1) * 512] = blk

    in_maps = []
    for core in range(8):
        b, g = divmod(core, 4)
        # local q head order: slab-major, (s, half) -> global head 8g + s + 4*half
        qheads = [8 * g + s + 4 * h for s in range(SLABS) for h in range(2)]
        kvheads = [2 * g, 2 * g + 1]

        wq_cols = np.concatenate([qh * HD + deint for qh in qheads])
        wk_cols = np.concatenate([kh * HD + deint for kh in kvheads])
        wv_cols = np.concatenate([np.arange(kh * HD, (kh + 1) * HD) for kh in kvheads])
        wo_rows = np.concatenate([np.arange(qh * HD, (qh + 1) * HD) for qh in qheads])

        in_maps.append({
            "xT": _to_mm_dtype(x[b].T),
            "wq": _to_mm_dtype(wq[:, wq_cols] * (1.0 / np.sqrt(HD))),
            "wk": _to_mm_dtype(wk[:, wk_cols]),
            "wv": _to_mm_dtype(wv[:, wv_cols]),
            "wo": _to_mm_dtype(wo[wo_rows, :]),
            "c4": _to_mm_dtype(c4),
            "s4": _to_mm_dtype(s4),
            "vones": _to_mm_dtype(vones),
            "msk": _to_mm_dtype(msk),
        })
    return in_maps


def kernel(x, freqs_cos, freqs_sin, wq, wk, wv, wo, _trace=False):
    from concourse.bass_utils import run_bass_kernel_spmd

    if "nc" not in _CACHE:
        _CACHE["nc"] = _build()
    nc = _CACHE["nc"]

    in_maps = _prep_inputs(x, freqs_cos, freqs_sin, wq, wk, wv, wo)
    res = run_bass_kernel_spmd(nc, in_maps, core_ids=list(range(8)), trace=_trace)
    _CACHE["last_result"] = res

    out = np.empty((B, T, DIM), dtype=np.float32)
    for b in range(B):
        acc = res.results[4 * b]["outT"].astype(np.float32)
        for g in range(1, 4):
            acc = acc + res.results[4 * b + g]["outT"].astype(np.float32)
        out[b] = acc.T
    return out


# revision 20
# speedup vs baseline: 1.0725x; 1.0725x over previous
"""GQA attention (B=2, T=2048, DIM=2048, NH=32, NKV=8, HD=64) with RoPE, causal,
on 8 TRN2 NeuronCores.

Sharding: data-parallel over B (2) x tensor-parallel over kv-head groups (4).
Core i handles batch i//4 and kv heads {2g, 2g+1} (g = i%4), i.e. q heads
8g..8g+8. wq/wk/wv column-parallel, wo row-parallel; host sums the 4 partial
outputs per batch.

Everything on-device is feature-major ("transposed"): x^T, Q^T, K^T are
[feature, t] so no on-device transposes are needed anywhere:
  QT[d,t] = wq^T x^T;  scoresT[s,q] = (KT slice)^T @ QT;  OT[d,q] = V^T @ PT;
  outT[o,t] = wo^T @ OT.  Host transposes the final [o,t] back to [t,o].

RoPE de-interleave: within each head the 64 features are permuted to
[32 real | 32 imag] (host permutes wq/wk columns), so rot = q*C + swap(q)*S
with the half-swap done by SBUF->SBUF DMA.

Attention inner loop (the key perf structure): for each 128-wide s-block, ONE
PSUM tile sc[128,1024] holds BOTH kv-head halves: cols 0:512 = scores of head
(slab,h0) against KT rows 0:64, cols 512:1024 = head (slab,h1) against KT rows
64:128.  The two K=64 score matmuls land on disjoint PE row groups (0,0) /
(64,0) and are emitted back-to-back with attention MMs at the highest
scheduling priority, so the hardware co-streams them (2x throughput for the
64-deep contraction).  One exp ACT instruction covers both halves; causal
masking multiplies a host-precomputed 0/1 tile only on the 4 diagonal s-blocks
of each q-chunk.  PV runs as two K=128 matmuls (one per half) accumulating
into per-half [65,512] PSUM tiles whose appended ones-column yields the
softmax denominator for free; 1/denom via reciprocal_approx_fast + gpsimd
partition_broadcast.

PSUM budget (8 banks): sc 2 bufs x 2 banks + ot_h0/ot_h1 1 buf x 1 bank each
+ proj/wo accumulator 2 bufs x 1 bank = 8.

Emission order per t-block: attention(tb) FIRST (highest priority, keeps the
co-stream pairs adjacent and PV launching right after exp), then proj(tb+1),
then wo(tb) as gap fillers for the ACT-paced exp stream.  All x tiles are
DMA-prefetched at kernel start across the sync/vector/scalar queues so the PE
never waits on input DMA mid-kernel.

All matmul operands are fp16 (KERNEL_MM_DTYPE also allows bf16/f32r); the PE
streams 2-byte operands at full 2.4 GHz rate, fp32 PSUM accumulate.
"""

import numpy as np

B, T, DIM = 2, 2048, 2048
NH, NKV, HD = 32, 8, 64
G = 4            # tensor-parallel groups
QH = NH // G     # 8 local q heads
SLABS = 4
KTILES = DIM // 128
TBLK = T // 512

_CACHE = {}
import os as _os
_MM_DTYPE = _os.environ.get("KERNEL_MM_DTYPE", "fp16")


def _to_mm_dtype(x: np.ndarray) -> np.ndarray:
    if _MM_DTYPE == "bf16":
        import ml_dtypes
        return np.ascontiguousarray(x, dtype=np.float32).astype(ml_dtypes.bfloat16)
    if _MM_DTYPE == "fp16":
        return np.ascontiguousarray(x, dtype=np.float32).astype(np.float16)
    return _round_f32r(x)


def _round_f32r(x: np.ndarray) -> np.ndarray:
    """Round f32 to the float32r grid (11 mantissa bits, round-to-nearest-even)."""
    x = np.ascontiguousarray(x, dtype=np.float32)
    xi = x.view(np.uint32).copy()
    shift = 12  # keep 11 mantissa bits
    lsb = (xi >> shift) & 1
    xi = (xi + ((1 << (shift - 1)) - 1) + lsb) & np.uint32(~((1 << shift) - 1) & 0xFFFFFFFF)
    return xi.view(np.float32)


def _build():
    import concourse.bass as bass
    import concourse.mybir as mybir
    import concourse.tile as tile
    from concourse import bacc

    F32 = mybir.dt.float32
    F32R = {"bf16": mybir.dt.bfloat16, "fp16": mybir.dt.float16,
            "f32r": mybir.dt.float32r}[_MM_DTYPE]
    EXP = mybir.ActivationFunctionType.Exp

    nc = bacc.Bacc("TRN2", target_bir_lowering=False, debug=False, num_devices=8)

    # host pre-reorders weights/x so each SBUF tile loads with ONE big DMA
    # (per-DMA overhead ~0.5-1us and same-tile writes serialize, so many
    # small loads gate the kernel start):
    #   xr[tb*128+p, k*512+t] = x^T[k*128+p, tb*512+t]
    #   wqr[s*128+p, k*128+c] = wq_cols[k*128+p, s*128+c] / sqrt(hd)
    #   wkr[p, k*128+c] = wk_cols[k*128+p, c]   (wvr likewise)
    xr = nc.dram_tensor("xr", [4 * 128, KTILES * 512], F32R, kind="ExternalInput").ap()
    wq = nc.dram_tensor("wq", [SLABS * 128, KTILES * 128], F32R, kind="ExternalInput").ap()
    wk = nc.dram_tensor("wk", [128, KTILES * 128], F32R, kind="ExternalInput").ap()
    wv = nc.dram_tensor("wv", [128, KTILES * 128], F32R, kind="ExternalInput").ap()
    wo = nc.dram_tensor("wo", [QH * HD, DIM], F32R, kind="ExternalInput").ap()
    c4 = nc.dram_tensor("c4", [128, T], F32R, kind="ExternalInput").ap()
    s4 = nc.dram_tensor("s4", [128, T], F32R, kind="ExternalInput").ap()
    vones = nc.dram_tensor("vones", [128, 16 * 65], F32R, kind="ExternalInput").ap()
    msk = nc.dram_tensor("msk", [128, 896], F32R, kind="ExternalInput").ap()
    outT = nc.dram_tensor("outT", [DIM, T], F32R, kind="ExternalOutput").ap()

    from contextlib import ExitStack

    with tile.TileContext(nc) as tc, ExitStack() as ctx:
        # ---------- persistent tiles ----------
        pers = ctx.enter_context(tc.tile_pool(name="pers", bufs=1))
        KT = pers.tile([128, T], F32R, tag="kt", name="kt")
        V0 = pers.tile([128, 16 * 65], F32R, tag="v0", name="v0")
        V1 = pers.tile([128, 16 * 65], F32R, tag="v1", name="v1")
        MSK = pers.tile([128, 896], F32R, tag="msk", name="msk_sb")
        C4 = pers.tile([128, T], F32R, tag="c4", name="c4_sb")
        S4 = pers.tile([128, T], F32R, tag="s4", name="s4_sb")
        WQs = [pers.tile([128, KTILES * 128], F32R, tag=f"wq{s}", name=f"wq{s}")
               for s in range(SLABS)]
        WK = pers.tile([128, KTILES * 128], F32R, tag="wk", name="wk_sb")
        WV = pers.tile([128, KTILES * 128], F32R, tag="wv", name="wv_sb")
        WO = [pers.tile([128, T], F32R, tag=f"wo{s}", name=f"wo{s}") for s in range(SLABS)]
        # all x resident for the whole kernel: XT[tb] = [128, k*512+t]
        XT = [pers.tile([128, KTILES * 512], F32R, tag=f"x{tb}", name=f"x{tb}")
              for tb in range(TBLK)]

        rot = ctx.enter_context(tc.tile_pool(name="rot", bufs=2))
        work = ctx.enter_context(tc.tile_pool(name="work", bufs=2))
        ptp = ctx.enter_context(tc.tile_pool(name="ptp", bufs=4))
        misc = ctx.enter_context(tc.tile_pool(name="misc", bufs=2))
        osbp = ctx.enter_context(tc.tile_pool(name="osbp", bufs=6))
        ps_acc = ctx.enter_context(tc.tile_pool(name="ps_acc", bufs=2, space="PSUM"))
        ps_sc = ctx.enter_context(tc.tile_pool(name="ps_sc", bufs=2, space="PSUM"))
        ps_ot = ctx.enter_context(tc.tile_pool(name="ps_ot", bufs=1, space="PSUM"))

        # ---------- input DMA: everything prefetched up front ----------
        # The gpsimd queue is reserved for the latency-critical rope swap
        # DMAs (tiny SBUF->SBUF transfers emitted inside emit_proj) -- bulk
        # loads queued ahead of them starve the rope chain and stall the PE.
        # Big contiguous DMAs (host pre-reordered), ordered by need-time.
        def xchunk(q, tb, c):  # quarter c of x(tb): k-tiles 4c..4c+3
            q.dma_start(XT[tb][:, c * 2048:(c + 1) * 2048],
                        xr[tb * 128:(tb + 1) * 128, c * 2048:(c + 1) * 2048])
        xchunk(nc.sync, 0, 0)
        xchunk(nc.sync, 0, 1)
        xchunk(nc.scalar, 0, 2)
        xchunk(nc.scalar, 0, 3)
        nc.scalar.dma_start(WQs[0][:], wq[0:128, :])
        nc.sync.dma_start(WV[:], wv[:])
        nc.sync.dma_start(WK[:], wk[:])
        nc.scalar.dma_start(C4[:, 0:512], c4[:, 0:512])
        nc.scalar.dma_start(S4[:, 0:512], s4[:, 0:512])
        nc.scalar.dma_start(MSK[:], msk[:])
        nc.sync.dma_start(WQs[1][:], wq[128:256, :])
        nc.scalar.dma_start(WQs[2][:], wq[256:384, :])
        nc.sync.dma_start(WQs[3][:], wq[384:512, :])
        for c in range(4):
            xchunk(nc.sync, 1, c)
        nc.scalar.dma_start(C4[:, 512:2048], c4[:, 512:2048])
        nc.scalar.dma_start(S4[:, 512:2048], s4[:, 512:2048])
        for s in range(SLABS):
            nc.sync.dma_start(WO[s][:], wo[s * 128:(s + 1) * 128, :])
        for c in range(4):
            xchunk(nc.scalar, 2, c)
        for c in range(4):
            xchunk(nc.scalar, 3, c)
        nc.gpsimd.dma_start(V0[:], vones[:])
        nc.gpsimd.dma_start(V1[:], vones[:])

        QTr_by_tb = {}

        def emit_proj(tb):
            t_sl = slice(tb * 512, (tb + 1) * 512)
            xt = XT[tb]
            QTr = [None] * SLABS

            def qk_group(s):
                # s < SLABS: q slab s; s == SLABS: k
                ps = ps_acc.tile([128, 512], F32, tag="acc", name="pq")
                for k in range(KTILES):
                    w = WQs[s] if s < SLABS else WK
                    nc.tensor.matmul(ps[:], w[:, k * 128:(k + 1) * 128],
                                     xt[:, k * 512:(k + 1) * 512],
                                     start=(k == 0), stop=(k == KTILES - 1))
                if s < SLABS:
                    dst_t = rot.tile([128, 512], F32R, tag=f"qtr{s}", name=f"qtr{s}")
                    QTr[s] = dst_t
                    dst = dst_t[:]
                else:
                    dst = KT[:, t_sl]
                # single PSUM read (frees the accumulator slot); rest of the
                # rope math is fp16 SBUF-only for the faster DVE tiers
                q_sb = work.tile([128, 512], F32R, tag="qsb", name="qsb")
                nc.vector.tensor_copy(q_sb[:], ps[:])
                q_sw = work.tile([128, 512], F32R, tag="qsw", name="qsw")
                for o in (0, 64):
                    nc.gpsimd.dma_start(q_sw[o:o + 32, :], q_sb[o + 32:o + 64, :])
                    nc.gpsimd.dma_start(q_sw[o + 32:o + 64, :], q_sb[o:o + 32, :])
                m1 = work.tile([128, 512], F32R, tag="m1", name="m1")
                nc.vector.tensor_mul(m1[:], q_sb[:], C4[:, t_sl])
                m2 = work.tile([128, 512], F32R, tag="m2", name="m2")
                nc.vector.tensor_mul(m2[:], q_sw[:], S4[:, t_sl])
                nc.vector.tensor_add(dst, m1[:], m2[:])

            def v_group(i):
                sbi = tb * 4 + i
                pv = ps_acc.tile([128, 128], F32, tag="acc", name="pv",
                                 padded_shape=[128, 512])
                for k in range(KTILES):
                    nc.tensor.matmul(pv[:], xt[:, k * 512 + i * 128: k * 512 + (i + 1) * 128],
                                     WV[:, k * 128:(k + 1) * 128],
                                     start=(k == 0), stop=(k == KTILES - 1))
                nc.vector.tensor_copy(V0[:, sbi * 65: sbi * 65 + 64], pv[:, 0:64])
                nc.vector.tensor_copy(V1[:, sbi * 65: sbi * 65 + 64], pv[:, 64:128])

            if tb == 0:
                # attention(0) needs qtr0 + kt + v blocks first; q2/q3 last
                for g in (0, 1, SLABS):
                    qk_group(g)
                for i in range(4):
                    v_group(i)
                qk_group(2)
                qk_group(3)
            else:
                # attention(tb) starts from s-block 0: qtr first, own k/v last
                for g in (0, 1, 2, 3, SLABS):
                    qk_group(g)
                for i in range(4):
                    v_group(i)
            QTr_by_tb[tb] = QTr

        def emit_attention(qc):
            QTr = QTr_by_tb.pop(qc)
            nblk = (qc + 1) * 4
            OTNr = []
            for s in range(SLABS):
                ot0 = ps_ot.tile([65, 512], F32, tag="ot0", name="ot0")
                ot1 = ps_ot.tile([65, 512], F32, tag="ot1", name="ot1")
                # PV matmuls are emitted one s-block BEHIND the score pair so
                # their scheduler priority is lower than the next score pair:
                # the co-streamed (row-group 0/64) score MMs stay adjacent in
                # the PE queue even when an exp completes mid-pair.
                pend = None

                def flush_pv(sb_):
                    nc.tensor.matmul(ot0[:], V0[:, sb_ * 65: sb_ * 65 + 65],
                                     pend[:, 0:512],
                                     start=(sb_ == 0), stop=(sb_ == nblk - 1))
                    nc.tensor.matmul(ot1[:], V1[:, sb_ * 65: sb_ * 65 + 65],
                                     pend[:, 512:1024],
                                     start=(sb_ == 0), stop=(sb_ == nblk - 1))

                for sb in range(nblk):
                    sc = ps_sc.tile([128, 1024], F32, tag="sc", name="sc")
                    nc.tensor.matmul(sc[:, 0:512],
                                     KT[0:64, sb * 128:(sb + 1) * 128],
                                     QTr[s][0:64, :], start=True, stop=True)
                    nc.tensor.matmul(sc[:, 512:1024],
                                     KT[64:128, sb * 128:(sb + 1) * 128],
                                     QTr[s][64:128, :], start=True, stop=True)
                    if sb > 0:
                        flush_pv(sb - 1)
                    pt = ptp.tile([128, 1024], F32R, tag="pt", name="pt")
                    nc.scalar.activation(pt[:], sc[:], EXP)
                    off = sb * 128 - qc * 512
                    if off >= 0:  # diagonal s-block: apply causal 0/1 mask
                        # MSK[p, j] = (p + 384 <= j); window a..a+512 gives
                        # the (128v + p) <= q mask; broadcast over both halves
                        a = 384 - off
                        pt3 = pt[:].rearrange("p (h q) -> p h q", h=2)
                        m3 = MSK[:, a:a + 512].unsqueeze(1).broadcast_to([128, 2, 512])
                        nc.vector.tensor_mul(pt3, pt3, m3)
                    pend = pt
                flush_pv(nblk - 1)
                otn = rot.tile([128, 512], F32R, tag=f"otnr{s}", name=f"otnr{s}")
                OTNr.append(otn)
                bcs = []
                for h, ot in ((0, ot0), (1, ot1)):
                    dsb = misc.tile([1, 512], F32, tag=f"dsb{h}", name=f"dsb{h}")
                    nc.vector.tensor_copy(dsb[:], ot[64:65, :])
                    rcf = misc.tile([1, 512], F32, tag=f"rcf{h}", name=f"rcf{h}")
                    nc.vector.reciprocal_approx_fast(rcf[:], dsb[:])
                    bc = misc.tile([64, 512], F32, tag=f"bc{h}", name=f"bc{h}")
                    nc.gpsimd.partition_broadcast(bc[:], rcf[:])
                    bcs.append(bc)
                nc.vector.tensor_mul(otn[0:64, :], ot0[0:64, :], bcs[0][:])
                nc.vector.tensor_mul(otn[64:128, :], ot1[0:64, :], bcs[1][:])
            return OTNr

        def emit_wo(tb, OTNr):
            t_sl = slice(tb * 512, (tb + 1) * 512)
            for ob in range(16):
                po = ps_acc.tile([128, 512], F32, tag="acc", name="po")
                for s in range(SLABS):
                    nc.tensor.matmul(po[:], WO[s][:, ob * 128:(ob + 1) * 128],
                                     OTNr[s][:], start=(s == 0), stop=(s == SLABS - 1))
                osb = osbp.tile([128, 512], F32R, tag="osb", name="osb")
                nc.vector.tensor_copy(osb[:], po[:])
                nc.sync.dma_start(outT[ob * 128:(ob + 1) * 128, t_sl], osb[:])

        emit_proj(0)
        for tb in range(TBLK):
            OTNr = emit_attention(tb)
            if tb + 1 < TBLK:
                emit_proj(tb + 1)
            emit_wo(tb, OTNr)

    nc.compile()
    return nc


def _prep_inputs(x, freqs_cos, freqs_sin, wq, wk, wv, wo):
    """Build the 8 per-core input maps (host-side sharding + layout prep)."""
    x = np.asarray(x, dtype=np.float32)
    freqs_cos = np.asarray(freqs_cos, dtype=np.float32)
    freqs_sin = np.asarray(freqs_sin, dtype=np.float32)
    wq = np.asarray(wq, dtype=np.float32)
    wk = np.asarray(wk, dtype=np.float32)
    wv = np.asarray(wv, dtype=np.float32)
    wo = np.asarray(wo, dtype=np.float32)

    # de-interleave permutation within a head: [2j] then [2j+1]
    deint = np.concatenate([np.arange(0, HD, 2), np.arange(1, HD, 2)])

    # rope tables [128, T]: row r uses freq index r % 32; sign of sin flips
    # per 32-block (real-out blocks get -sin)
    cosT = freqs_cos.T  # [32, T]
    sinT = freqs_sin.T
    c4 = np.tile(cosT, (4, 1)).astype(np.float32)
    s4 = np.concatenate([-sinT, sinT, -sinT, sinT], axis=0).astype(np.float32)

    vones = np.zeros((128, 16 * 65), dtype=np.float32)
    vones[:, 64::65] = 1.0
    # sliding-window causal mask: msk[p, j] = 1 iff (p + 384) <= j; the
    # diagonal s-block at offset off=128v uses window cols (384-off)..+512
    msk = ((np.arange(128)[:, None] + 384) <= np.arange(896)[None, :]).astype(np.float32)

    def relayout_kp(w, inner):
        # [k*128+p, c] rows -> [p, k*inner+c] (k-tiles side by side)
        return w.reshape(KTILES, 128, inner).transpose(1, 0, 2).reshape(128, KTILES * inner)

    in_maps = []
    for core in range(8):
        b, g = divmod(core, 4)
        # local q head order: slab-major, (s, half) -> global head 8g + s + 4*half
        qheads = [8 * g + s + 4 * h for s in range(SLABS) for h in range(2)]
        kvheads = [2 * g, 2 * g + 1]

        wq_cols = np.concatenate([qh * HD + deint for qh in qheads])
        wk_cols = np.concatenate([kh * HD + deint for kh in kvheads])
        wv_cols = np.concatenate([np.arange(kh * HD, (kh + 1) * HD) for kh in kvheads])
        wo_rows = np.concatenate([np.arange(qh * HD, (qh + 1) * HD) for qh in qheads])

        # xr[tb*128+p, k*512+t] = x^T[k*128+p, tb*512+t]
        xr = (x[b].T.reshape(KTILES, 128, TBLK, 512)
              .transpose(2, 1, 0, 3).reshape(TBLK * 128, KTILES * 512))
        # wqr[s*128+p, k*128+c] = wq_sel[k*128+p, s*128+c]
        wq_sel = wq[:, wq_cols] * (1.0 / np.sqrt(HD))
        wqr = (wq_sel.reshape(KTILES, 128, SLABS, 128)
               .transpose(2, 1, 0, 3).reshape(SLABS * 128, KTILES * 128))

        in_maps.append({
            "xr": _to_mm_dtype(xr),
            "wq": _to_mm_dtype(wqr),
            "wk": _to_mm_dtype(relayout_kp(wk[:, wk_cols], 128)),
            "wv": _to_mm_dtype(relayout_kp(wv[:, wv_cols], 128)),
            "wo": _to_mm_dtype(wo[wo_rows, :]),
            "c4": _to_mm_dtype(c4),
            "s4": _to_mm_dtype(s4),
            "vones": _to_mm_dtype(vones),
            "msk": _to_mm_dtype(msk),
        })
    return in_maps


def kernel(x, freqs_cos, freqs_sin, wq, wk, wv, wo, _trace=False):
    from concourse.bass_utils import run_bass_kernel_spmd

    if "nc" not in _CACHE:
        _CACHE["nc"] = _build()
    nc = _CACHE["nc"]

    in_maps = _prep_inputs(x, freqs_cos, freqs_sin, wq, wk, wv, wo)
    res = run_bass_kernel_spmd(nc, in_maps, core_ids=list(range(8)), trace=_trace)
    _CACHE["last_result"] = res

    out = np.empty((B, T, DIM), dtype=np.float32)
    for b in range(B):
        acc = res.results[4 * b]["outT"].astype(np.float32)
        for g in range(1, 4):
            acc = acc + res.results[4 * b + g]["outT"].astype(np.float32)
        out[b] = acc.T
    return out


# revision 23
# speedup vs baseline: 1.0750x; 1.0024x over previous
"""GQA attention (B=2, T=2048, DIM=2048, NH=32, NKV=8, HD=64) with RoPE, causal,
on 8 TRN2 NeuronCores.

Sharding: data-parallel over B (2) x tensor-parallel over kv-head groups (4).
Core i handles batch i//4 and kv heads {2g, 2g+1} (g = i%4), i.e. q heads
8g..8g+8. wq/wk/wv column-parallel, wo row-parallel; host sums the 4 partial
outputs per batch.

Everything on-device is feature-major ("transposed"): x^T, Q^T, K^T are
[feature, t] so no on-device transposes are needed anywhere:
  QT[d,t] = wq^T x^T;  scoresT[s,q] = (KT slice)^T @ QT;  OT[d,q] = V^T @ PT;
  outT[o,t] = wo^T @ OT.  Host transposes the final [o,t] back to [t,o].

RoPE de-interleave: within each head the 64 features are permuted to
[32 real | 32 imag] (host permutes wq/wk columns), so rot = q*C + swap(q)*S
with the half-swap done by SBUF->SBUF DMA.

Attention inner loop (the key perf structure): for each 128-wide s-block, ONE
PSUM tile sc[128,1024] holds BOTH kv-head halves: cols 0:512 = scores of head
(slab,h0) against KT rows 0:64, cols 512:1024 = head (slab,h1) against KT rows
64:128.  The two K=64 score matmuls land on disjoint PE row groups (0,0) /
(64,0) and are emitted back-to-back with attention MMs at the highest
scheduling priority, so the hardware co-streams them (2x throughput for the
64-deep contraction).  One exp ACT instruction covers both halves; causal
masking multiplies a host-precomputed 0/1 tile only on the 4 diagonal s-blocks
of each q-chunk.  PV runs as two K=128 matmuls (one per half) accumulating
into per-half [65,512] PSUM tiles whose appended ones-column yields the
softmax denominator for free; 1/denom via reciprocal_approx_fast + gpsimd
partition_broadcast.

PSUM budget (8 banks): sc 2 bufs x 2 banks + ot_h0/ot_h1 1 buf x 1 bank each
+ proj/wo accumulator 2 bufs x 1 bank = 8.

Emission order per t-block: attention(tb) FIRST (highest priority, keeps the
co-stream pairs adjacent and PV launching right after exp), then proj(tb+1),
then wo(tb) as gap fillers for the ACT-paced exp stream.  All x tiles are
DMA-prefetched at kernel start across the sync/vector/scalar queues so the PE
never waits on input DMA mid-kernel.

All matmul operands are fp16 (KERNEL_MM_DTYPE also allows bf16/f32r); the PE
streams 2-byte operands at full 2.4 GHz rate, fp32 PSUM accumulate.
"""

import numpy as np

B, T, DIM = 2, 2048, 2048
NH, NKV, HD = 32, 8, 64
G = 4            # tensor-parallel groups
QH = NH // G     # 8 local q heads
SLABS = 4
KTILES = DIM // 128
TBLK = T // 512

_CACHE = {}
import os as _os
_MM_DTYPE = _os.environ.get("KERNEL_MM_DTYPE", "fp16")


def _to_mm_dtype(x: np.ndarray) -> np.ndarray:
    if _MM_DTYPE == "bf16":
        import ml_dtypes
        return np.ascontiguousarray(x, dtype=np.float32).astype(ml_dtypes.bfloat16)
    if _MM_DTYPE == "fp16":
        return np.ascontiguousarray(x, dtype=np.float32).astype(np.float16)
    return _round_f32r(x)


def _round_f32r(x: np.ndarray) -> np.ndarray:
    """Round f32 to the float32r grid (11 mantissa bits, round-to-nearest-even)."""
    x = np.ascontiguousarray(x, dtype=np.float32)
    xi = x.view(np.uint32).copy()
    shift = 12  # keep 11 mantissa bits
    lsb = (xi >> shift) & 1
    xi = (xi + ((1 << (shift - 1)) - 1) + lsb) & np.uint32(~((1 << shift) - 1) & 0xFFFFFFFF)
    return xi.view(np.float32)


def _build():
    import concourse.bass as bass
    import concourse.mybir as mybir
    import concourse.tile as tile
    from concourse import bacc

    F32 = mybir.dt.float32
    F32R = {"bf16": mybir.dt.bfloat16, "fp16": mybir.dt.float16,
            "f32r": mybir.dt.float32r}[_MM_DTYPE]
    EXP = mybir.ActivationFunctionType.Exp

    nc = bacc.Bacc("TRN2", target_bir_lowering=False, debug=False, num_devices=8)

    # host pre-reorders weights/x so each SBUF tile loads with ONE big DMA
    # (per-DMA overhead ~0.5-1us and same-tile writes serialize, so many
    # small loads gate the kernel start):
    #   xr[tb*128+p, k*512+t] = x^T[k*128+p, tb*512+t]
    #   wqr[s*128+p, k*128+c] = wq_cols[k*128+p, s*128+c] / sqrt(hd)
    #   wkr[p, k*128+c] = wk_cols[k*128+p, c]   (wvr likewise)
    xr = nc.dram_tensor("xr", [4 * 128, KTILES * 512], F32R, kind="ExternalInput").ap()
    wq = nc.dram_tensor("wq", [SLABS * 128, KTILES * 128], F32R, kind="ExternalInput").ap()
    wk = nc.dram_tensor("wk", [128, KTILES * 128], F32R, kind="ExternalInput").ap()
    wv = nc.dram_tensor("wv", [128, KTILES * 128], F32R, kind="ExternalInput").ap()
    wo = nc.dram_tensor("wo", [QH * HD, DIM], F32R, kind="ExternalInput").ap()
    c4 = nc.dram_tensor("c4", [128, T], F32R, kind="ExternalInput").ap()
    s4 = nc.dram_tensor("s4", [128, T], F32R, kind="ExternalInput").ap()
    vones = nc.dram_tensor("vones", [128, 16 * 65], F32R, kind="ExternalInput").ap()
    msk = nc.dram_tensor("msk", [128, 896], F32R, kind="ExternalInput").ap()
    outT = nc.dram_tensor("outT", [DIM, T], F32R, kind="ExternalOutput").ap()

    from contextlib import ExitStack

    with tile.TileContext(nc) as tc, ExitStack() as ctx:
        # ---------- persistent tiles ----------
        pers = ctx.enter_context(tc.tile_pool(name="pers", bufs=1))
        KT = pers.tile([128, T], F32R, tag="kt", name="kt")
        V0 = pers.tile([128, 16 * 65], F32R, tag="v0", name="v0")
        V1 = pers.tile([128, 16 * 65], F32R, tag="v1", name="v1")
        MSK = pers.tile([128, 896], F32R, tag="msk", name="msk_sb")
        C4 = pers.tile([128, T], F32R, tag="c4", name="c4_sb")
        S4 = pers.tile([128, T], F32R, tag="s4", name="s4_sb")
        WQs = [pers.tile([128, KTILES * 128], F32R, tag=f"wq{s}", name=f"wq{s}")
               for s in range(SLABS)]
        WK = pers.tile([128, KTILES * 128], F32R, tag="wk", name="wk_sb")
        WV = pers.tile([128, KTILES * 128], F32R, tag="wv", name="wv_sb")
        WO = [pers.tile([128, T], F32R, tag=f"wo{s}", name=f"wo{s}") for s in range(SLABS)]
        # all x resident for the whole kernel: XT[tb] = [128, k*512+t]
        XT = [pers.tile([128, KTILES * 512], F32R, tag=f"x{tb}", name=f"x{tb}")
              for tb in range(TBLK)]

        rot = ctx.enter_context(tc.tile_pool(name="rot", bufs=2))
        work = ctx.enter_context(tc.tile_pool(name="work", bufs=2))
        ptp = ctx.enter_context(tc.tile_pool(name="ptp", bufs=4))
        misc = ctx.enter_context(tc.tile_pool(name="misc", bufs=2))
        osbp = ctx.enter_context(tc.tile_pool(name="osbp", bufs=6))
        ps_acc = ctx.enter_context(tc.tile_pool(name="ps_acc", bufs=2, space="PSUM"))
        ps_sc = ctx.enter_context(tc.tile_pool(name="ps_sc", bufs=2, space="PSUM"))
        ps_ot = ctx.enter_context(tc.tile_pool(name="ps_ot", bufs=1, space="PSUM"))

        # ---------- input DMA: everything prefetched up front ----------
        # The gpsimd queue is reserved for the latency-critical rope swap
        # DMAs (tiny SBUF->SBUF transfers emitted inside emit_proj) -- bulk
        # loads queued ahead of them starve the rope chain and stall the PE.
        # Big contiguous DMAs (host pre-reordered), ordered by need-time.
        def xchunk(q, tb, c):  # quarter c of x(tb): k-tiles 4c..4c+3
            q.dma_start(XT[tb][:, c * 2048:(c + 1) * 2048],
                        xr[tb * 128:(tb + 1) * 128, c * 2048:(c + 1) * 2048])
        nc.scalar.dma_start(WQs[0][:], wq[0:128, :])
        xchunk(nc.sync, 0, 0)
        xchunk(nc.sync, 0, 1)
        xchunk(nc.scalar, 0, 2)
        xchunk(nc.scalar, 0, 3)
        nc.sync.dma_start(WK[:], wk[:])
        nc.sync.dma_start(WV[:], wv[:])
        nc.scalar.dma_start(C4[:, 0:512], c4[:, 0:512])
        nc.scalar.dma_start(S4[:, 0:512], s4[:, 0:512])
        nc.scalar.dma_start(MSK[:], msk[:])
        nc.sync.dma_start(WQs[1][:], wq[128:256, :])
        nc.scalar.dma_start(WQs[2][:], wq[256:384, :])
        nc.sync.dma_start(WQs[3][:], wq[384:512, :])
        for c in range(4):
            xchunk(nc.sync, 1, c)
        nc.scalar.dma_start(C4[:, 512:2048], c4[:, 512:2048])
        nc.scalar.dma_start(S4[:, 512:2048], s4[:, 512:2048])
        for s in range(SLABS):
            nc.sync.dma_start(WO[s][:], wo[s * 128:(s + 1) * 128, :])
        for c in range(4):
            xchunk(nc.scalar, 2, c)
        for c in range(4):
            xchunk(nc.scalar, 3, c)
        nc.gpsimd.dma_start(V0[:], vones[:])
        nc.gpsimd.dma_start(V1[:], vones[:])

        QTr_by_tb = {}

        def emit_proj(tb):
            t_sl = slice(tb * 512, (tb + 1) * 512)
            xt = XT[tb]
            QTr = [None] * SLABS

            # rope math for group g is emitted only after group g+1's PSUM
            # copy: the copies (which free the accumulator slots) then never
            # queue on the DVE behind swap-DMA-dependent muls.
            lag = []

            def qk_front(s):
                # s < SLABS: q slab s; s == SLABS: k
                ps = ps_acc.tile([128, 512], F32, tag="acc", name="pq")
                for k in range(KTILES):
                    w = WQs[s] if s < SLABS else WK
                    nc.tensor.matmul(ps[:], w[:, k * 128:(k + 1) * 128],
                                     xt[:, k * 512:(k + 1) * 512],
                                     start=(k == 0), stop=(k == KTILES - 1))
                if s < SLABS:
                    dst_t = rot.tile([128, 512], F32R, tag=f"qtr{s}", name=f"qtr{s}")
                    QTr[s] = dst_t
                    dst = dst_t[:]
                else:
                    dst = KT[:, t_sl]
                # single PSUM read (frees the accumulator slot); rest of the
                # rope math is fp16 SBUF-only for the faster DVE tiers
                q_sb = work.tile([128, 512], F32R, tag="qsb", name="qsb")
                nc.vector.tensor_copy(q_sb[:], ps[:])
                q_sw = work.tile([128, 512], F32R, tag="qsw", name="qsw")
                for o in (0, 64):
                    nc.gpsimd.dma_start(q_sw[o:o + 32, :], q_sb[o + 32:o + 64, :])
                    nc.gpsimd.dma_start(q_sw[o + 32:o + 64, :], q_sb[o:o + 32, :])
                lag.append((q_sb, q_sw, dst))
                if len(lag) > 1:
                    rope_math(lag.pop(0))

            def rope_math(st):
                q_sb, q_sw, dst = st
                m1 = work.tile([128, 512], F32R, tag="m1", name="m1")
                nc.vector.tensor_mul(m1[:], q_sb[:], C4[:, t_sl])
                m2 = work.tile([128, 512], F32R, tag="m2", name="m2")
                nc.vector.tensor_mul(m2[:], q_sw[:], S4[:, t_sl])
                nc.vector.tensor_add(dst, m1[:], m2[:])

            def rope_flush():
                while lag:
                    rope_math(lag.pop(0))

            def v_group(i):
                sbi = tb * 4 + i
                pv = ps_acc.tile([128, 128], F32, tag="acc", name="pv",
                                 padded_shape=[128, 512])
                for k in range(KTILES):
                    nc.tensor.matmul(pv[:], xt[:, k * 512 + i * 128: k * 512 + (i + 1) * 128],
                                     WV[:, k * 128:(k + 1) * 128],
                                     start=(k == 0), stop=(k == KTILES - 1))
                nc.vector.tensor_copy(V0[:, sbi * 65: sbi * 65 + 64], pv[:, 0:64])
                nc.vector.tensor_copy(V1[:, sbi * 65: sbi * 65 + 64], pv[:, 64:128])

            if tb == 0:
                # attention(0) needs qtr0 + kt + v blocks first; q2/q3 last
                qk_front(0)
                qk_front(SLABS)   # flushes rope(q0)
                qk_front(1)       # flushes rope(k) -> KT ready
                v_group(0)
                rope_flush()      # rope(q1)
                for i in range(1, 4):
                    v_group(i)
                qk_front(2)
                qk_front(3)       # flushes rope(q2)
                rope_flush()
            else:
                # attention(tb) starts from s-block 0: qtr first, own k/v last
                for g in (0, 1, 2, 3, SLABS):
                    qk_front(g)
                rope_flush()
                for i in range(4):
                    v_group(i)
            QTr_by_tb[tb] = QTr

        def emit_attention(qc):
            QTr = QTr_by_tb.pop(qc)
            nblk = (qc + 1) * 4
            OTNr = []
            for s in range(SLABS):
                ot0 = ps_ot.tile([65, 512], F32, tag="ot0", name="ot0")
                ot1 = ps_ot.tile([65, 512], F32, tag="ot1", name="ot1")
                # PV matmuls are emitted one s-block BEHIND the score pair so
                # their scheduler priority is lower than the next score pair:
                # the co-streamed (row-group 0/64) score MMs stay adjacent in
                # the PE queue even when an exp completes mid-pair.
                pend = None

                def flush_pv(sb_):
                    nc.tensor.matmul(ot0[:], V0[:, sb_ * 65: sb_ * 65 + 65],
                                     pend[:, 0:512],
                                     start=(sb_ == 0), stop=(sb_ == nblk - 1))
                    nc.tensor.matmul(ot1[:], V1[:, sb_ * 65: sb_ * 65 + 65],
                                     pend[:, 512:1024],
                                     start=(sb_ == 0), stop=(sb_ == nblk - 1))

                for sb in range(nblk):
                    sc = ps_sc.tile([128, 1024], F32, tag="sc", name="sc")
                    nc.tensor.matmul(sc[:, 0:512],
                                     KT[0:64, sb * 128:(sb + 1) * 128],
                                     QTr[s][0:64, :], start=True, stop=True)
                    nc.tensor.matmul(sc[:, 512:1024],
                                     KT[64:128, sb * 128:(sb + 1) * 128],
                                     QTr[s][64:128, :], start=True, stop=True)
                    if sb > 0:
                        flush_pv(sb - 1)
                    pt = ptp.tile([128, 1024], F32R, tag="pt", name="pt")
                    nc.scalar.activation(pt[:], sc[:], EXP)
                    off = sb * 128 - qc * 512
                    if off >= 0:  # diagonal s-block: apply causal 0/1 mask
                        # MSK[p, j] = (p + 384 <= j); window a..a+512 gives
                        # the (128v + p) <= q mask; broadcast over both halves
                        a = 384 - off
                        pt3 = pt[:].rearrange("p (h q) -> p h q", h=2)
                        m3 = MSK[:, a:a + 512].unsqueeze(1).broadcast_to([128, 2, 512])
                        nc.vector.tensor_mul(pt3, pt3, m3)
                    pend = pt
                flush_pv(nblk - 1)
                otn = rot.tile([128, 512], F32R, tag=f"otnr{s}", name=f"otnr{s}")
                OTNr.append(otn)
                bcs = []
                for h, ot in ((0, ot0), (1, ot1)):
                    dsb = misc.tile([1, 512], F32, tag=f"dsb{h}", name=f"dsb{h}")
                    nc.vector.tensor_copy(dsb[:], ot[64:65, :])
                    rcf = misc.tile([1, 512], F32, tag=f"rcf{h}", name=f"rcf{h}")
                    nc.vector.reciprocal_approx_fast(rcf[:], dsb[:])
                    bc = misc.tile([64, 512], F32, tag=f"bc{h}", name=f"bc{h}")
                    nc.gpsimd.partition_broadcast(bc[:], rcf[:])
                    bcs.append(bc)
                nc.vector.tensor_mul(otn[0:64, :], ot0[0:64, :], bcs[0][:])
                nc.vector.tensor_mul(otn[64:128, :], ot1[0:64, :], bcs[1][:])
            return OTNr

        def emit_wo(tb, OTNr):
            t_sl = slice(tb * 512, (tb + 1) * 512)
            for ob in range(16):
                po = ps_acc.tile([128, 512], F32, tag="acc", name="po")
                for s in range(SLABS):
                    nc.tensor.matmul(po[:], WO[s][:, ob * 128:(ob + 1) * 128],
                                     OTNr[s][:], start=(s == 0), stop=(s == SLABS - 1))
                osb = osbp.tile([128, 512], F32R, tag="osb", name="osb")
                nc.vector.tensor_copy(osb[:], po[:])
                nc.sync.dma_start(outT[ob * 128:(ob + 1) * 128, t_sl], osb[:])

        emit_proj(0)
        for tb in range(TBLK):
            OTNr = emit_attention(tb)
            if tb + 1 < TBLK:
                emit_proj(tb + 1)
            emit_wo(tb, OTNr)

    nc.compile()
    return nc


def _prep_inputs(x, freqs_cos, freqs_sin, wq, wk, wv, wo):
    """Build the 8 per-core input maps (host-side sharding + layout prep)."""
    x = np.asarray(x, dtype=np.float32)
    freqs_cos = np.asarray(freqs_cos, dtype=np.float32)
    freqs_sin = np.asarray(freqs_sin, dtype=np.float32)
    wq = np.asarray(wq, dtype=np.float32)
    wk = np.asarray(wk, dtype=np.float32)
    wv = np.asarray(wv, dtype=np.float32)
    wo = np.asarray(wo, dtype=np.float32)

    # de-interleave permutation within a head: [2j] then [2j+1]
    deint = np.concatenate([np.arange(0, HD, 2), np.arange(1, HD, 2)])

    # rope tables [128, T]: row r uses freq index r % 32; sign of sin flips
    # per 32-block (real-out blocks get -sin)
    cosT = freqs_cos.T  # [32, T]
    sinT = freqs_sin.T
    c4 = np.tile(cosT, (4, 1)).astype(np.float32)
    s4 = np.concatenate([-sinT, sinT, -sinT, sinT], axis=0).astype(np.float32)

    vones = np.zeros((128, 16 * 65), dtype=np.float32)
    vones[:, 64::65] = 1.0
    # sliding-window causal mask: msk[p, j] = 1 iff (p + 384) <= j; the
    # diagonal s-block at offset off=128v uses window cols (384-off)..+512
    msk = ((np.arange(128)[:, None] + 384) <= np.arange(896)[None, :]).astype(np.float32)

    def relayout_kp(w, inner):
        # [k*128+p, c] rows -> [p, k*inner+c] (k-tiles side by side)
        return w.reshape(KTILES, 128, inner).transpose(1, 0, 2).reshape(128, KTILES * inner)

    in_maps = []
    for core in range(8):
        b, g = divmod(core, 4)
        # local q head order: slab-major, (s, half) -> global head 8g + s + 4*half
        qheads = [8 * g + s + 4 * h for s in range(SLABS) for h in range(2)]
        kvheads = [2 * g, 2 * g + 1]

        wq_cols = np.concatenate([qh * HD + deint for qh in qheads])
        wk_cols = np.concatenate([kh * HD + deint for kh in kvheads])
        wv_cols = np.concatenate([np.arange(kh * HD, (kh + 1) * HD) for kh in kvheads])
        wo_rows = np.concatenate([np.arange(qh * HD, (qh + 1) * HD) for qh in qheads])

        # xr[tb*128+p, k*512+t] = x^T[k*128+p, tb*512+t]
        xr = (x[b].T.reshape(KTILES, 128, TBLK, 512)
              .transpose(2, 1, 0, 3).reshape(TBLK * 128, KTILES * 512))
        # wqr[s*128+p, k*128+c] = wq_sel[k*128+p, s*128+c]
        wq_sel = wq[:, wq_cols] * (1.0 / np.sqrt(HD))
        wqr = (wq_sel.reshape(KTILES, 128, SLABS, 128)
               .transpose(2, 1, 0, 3).reshape(SLABS * 128, KTILES * 128))

        in_maps.append({
            "xr": _to_mm_dtype(xr),
            "wq": _to_mm_dtype(wqr),
            "wk": _to_mm_dtype(relayout_kp(wk[:, wk_cols], 128)),
            "wv": _to_mm_dtype(relayout_kp(wv[:, wv_cols], 128)),
            "wo": _to_mm_dtype(wo[wo_rows, :]),
            "c4": _to_mm_dtype(c4),
            "s4": _to_mm_dtype(s4),
            "vones": _to_mm_dtype(vones),
            "msk": _to_mm_dtype(msk),
        })
    return in_maps


def kernel(x, freqs_cos, freqs_sin, wq, wk, wv, wo, _trace=False):
    from concourse.bass_utils import run_bass_kernel_spmd

    if "nc" not in _CACHE:
        _CACHE["nc"] = _build()
    nc = _CACHE["nc"]

    in_maps = _prep_inputs(x, freqs_cos, freqs_sin, wq, wk, wv, wo)
    res = run_bass_kernel_spmd(nc, in_maps, core_ids=list(range(8)), trace=_trace)
    _CACHE["last_result"] = res

    out = np.empty((B, T, DIM), dtype=np.float32)
    for b in range(B):
        acc = res.results[4 * b]["outT"].astype(np.float32)
        for g in range(1, 4):
            acc = acc + res.results[4 * b + g]["outT"].astype(np.float32)
        out[b] = acc.T
    return out


# revision 29
# speedup vs baseline: 1.1394x; 1.0599x over previous
"""GQA attention (B=2, T=2048, DIM=2048, NH=32, NKV=8, HD=64) with RoPE, causal,
on 8 TRN2 NeuronCores.

Sharding: data-parallel over B (2) x tensor-parallel over kv-head groups (4).
Core i handles batch i//4 and kv heads {2g, 2g+1} (g = i%4), i.e. q heads
8g..8g+8. wq/wk/wv column-parallel, wo row-parallel; host sums the 4 partial
outputs per batch.

Everything on-device is feature-major ("transposed"): x^T, Q^T, K^T are
[feature, t] so no on-device transposes are needed anywhere:
  QT[d,t] = wq^T x^T;  scoresT[s,q] = (KT slice)^T @ QT;  OT[d,q] = V^T @ PT;
  outT[o,t] = wo^T @ OT.  Host transposes the final [o,t] back to [t,o].

RoPE de-interleave: within each head the 64 features are permuted to
[re 0:16 | im 0:16 | re 16:32 | im 16:32] (host permutes wq/wk columns), so
rot = q*C + swap(q)*S where the +-16-partition swap is a single DVE
stream_shuffle (each 32-partition shuffle quadrant holds a re/im block pair).

Attention inner loop (the key perf structure): for each 128-wide s-block, ONE
PSUM tile sc[128,1024] holds BOTH kv-head halves: cols 0:512 = scores of head
(slab,h0) against KT rows 0:64, cols 512:1024 = head (slab,h1) against KT rows
64:128.  The two K=64 score matmuls land on disjoint PE row groups (0,0) /
(64,0) and are emitted back-to-back with attention MMs at the highest
scheduling priority, so the hardware co-streams them (2x throughput for the
64-deep contraction).  One exp ACT instruction covers both halves (clipped to
the causally-live columns on diagonal s-blocks); causal masking multiplies a
sliding-window 0/1 tile only on the 4 diagonal s-blocks of each q-chunk.  PV
runs as two K=128 matmuls (one per half) accumulating into per-half [128,512]
PSUM tiles; V tiles are padded to 128 cols/s-block (FWL-eligible ldweights)
with an appended ones-column that yields the softmax denominator for free;
1/denom via reciprocal_approx_fast + gpsimd partition_broadcast.

PSUM budget (8 banks): sc 2 bufs x 2 banks + ot_h0/ot_h1 1 buf x 1 bank each
+ proj/wo accumulator 2 bufs x 1 bank = 8.

Emission order per t-block: attention(tb) FIRST (highest priority, keeps the
co-stream pairs adjacent and PV launching right after exp), then proj(tb+1),
then wo(tb) as gap fillers for the ACT-paced exp stream.  All x tiles are
DMA-prefetched at kernel start across the sync/vector/scalar queues so the PE
never waits on input DMA mid-kernel.

All matmul operands are fp16 (KERNEL_MM_DTYPE also allows bf16/f32r); the PE
streams 2-byte operands at full 2.4 GHz rate, fp32 PSUM accumulate.
"""

import numpy as np

B, T, DIM = 2, 2048, 2048
NH, NKV, HD = 32, 8, 64
G = 4            # tensor-parallel groups
QH = NH // G     # 8 local q heads
SLABS = 4
KTILES = DIM // 128
TBLK = T // 512

_CACHE = {}
import os as _os
_MM_DTYPE = _os.environ.get("KERNEL_MM_DTYPE", "fp16")


def _to_mm_dtype(x: np.ndarray) -> np.ndarray:
    if _MM_DTYPE == "bf16":
        import ml_dtypes
        return np.ascontiguousarray(x, dtype=np.float32).astype(ml_dtypes.bfloat16)
    if _MM_DTYPE == "fp16":
        return np.ascontiguousarray(x, dtype=np.float32).astype(np.float16)
    return _round_f32r(x)


def _round_f32r(x: np.ndarray) -> np.ndarray:
    """Round f32 to the float32r grid (11 mantissa bits, round-to-nearest-even)."""
    x = np.ascontiguousarray(x, dtype=np.float32)
    xi = x.view(np.uint32).copy()
    shift = 12  # keep 11 mantissa bits
    lsb = (xi >> shift) & 1
    xi = (xi + ((1 << (shift - 1)) - 1) + lsb) & np.uint32(~((1 << shift) - 1) & 0xFFFFFFFF)
    return xi.view(np.float32)


def _build():
    import concourse.bass as bass
    import concourse.mybir as mybir
    import concourse.tile as tile
    from concourse import bacc

    F32 = mybir.dt.float32
    F32R = {"bf16": mybir.dt.bfloat16, "fp16": mybir.dt.float16,
            "f32r": mybir.dt.float32r}[_MM_DTYPE]
    EXP = mybir.ActivationFunctionType.Exp

    nc = bacc.Bacc("TRN2", target_bir_lowering=False, debug=False, num_devices=8)

    # host pre-reorders weights/x so each SBUF tile loads with ONE big DMA
    # (per-DMA overhead ~0.5-1us and same-tile writes serialize, so many
    # small loads gate the kernel start):
    #   xr[tb*128+p, k*512+t] = x^T[k*128+p, tb*512+t]
    #   wqr[s*128+p, k*128+c] = wq_cols[k*128+p, s*128+c] / sqrt(hd)
    #   wkr[p, k*128+c] = wk_cols[k*128+p, c]   (wvr likewise)
    xr = nc.dram_tensor("xr", [4 * 128, KTILES * 512], F32R, kind="ExternalInput").ap()
    wq = nc.dram_tensor("wq", [SLABS * 128, KTILES * 128], F32R, kind="ExternalInput").ap()
    wk = nc.dram_tensor("wk", [128, KTILES * 128], F32R, kind="ExternalInput").ap()
    wv = nc.dram_tensor("wv", [128, KTILES * 128], F32R, kind="ExternalInput").ap()
    wo = nc.dram_tensor("wo", [QH * HD, DIM], F32R, kind="ExternalInput").ap()
    c4 = nc.dram_tensor("c4", [128, T], F32R, kind="ExternalInput").ap()
    s4 = nc.dram_tensor("s4", [128, T], F32R, kind="ExternalInput").ap()
    vones = nc.dram_tensor("vones", [128, 16 * 128], F32R, kind="ExternalInput").ap()
    msk = nc.dram_tensor("msk", [128, 896], F32R, kind="ExternalInput").ap()
    outT = nc.dram_tensor("outT", [DIM, T], F32R, kind="ExternalOutput").ap()

    from contextlib import ExitStack

    with tile.TileContext(nc) as tc, ExitStack() as ctx:
        # ---------- persistent tiles ----------
        pers = ctx.enter_context(tc.tile_pool(name="pers", bufs=1))
        KT = pers.tile([128, T], F32R, tag="kt", name="kt")
        # v tiles padded to 128 cols per s-block (cols 65:128 zero) so the
        # pv ldweights is a full 128-col load (fast-weight-load eligible)
        V0 = pers.tile([128, 16 * 128], F32R, tag="v0", name="v0")
        V1 = pers.tile([128, 16 * 128], F32R, tag="v1", name="v1")
        MSK = pers.tile([128, 896], F32R, tag="msk", name="msk_sb")
        C4 = pers.tile([128, T], F32R, tag="c4", name="c4_sb")
        S4 = pers.tile([128, T], F32R, tag="s4", name="s4_sb")
        WQs = [pers.tile([128, KTILES * 128], F32R, tag=f"wq{s}", name=f"wq{s}")
               for s in range(SLABS)]
        WK = pers.tile([128, KTILES * 128], F32R, tag="wk", name="wk_sb")
        WV = pers.tile([128, KTILES * 128], F32R, tag="wv", name="wv_sb")
        WO = [pers.tile([128, T], F32R, tag=f"wo{s}", name=f"wo{s}") for s in range(SLABS)]
        # all x resident for the whole kernel: XT[tb] = [128, k*512+t]
        XT = [pers.tile([128, KTILES * 512], F32R, tag=f"x{tb}", name=f"x{tb}")
              for tb in range(TBLK)]

        rot = ctx.enter_context(tc.tile_pool(name="rot", bufs=2))
        work = ctx.enter_context(tc.tile_pool(name="work", bufs=2))
        ptp = ctx.enter_context(tc.tile_pool(name="ptp", bufs=4))
        misc = ctx.enter_context(tc.tile_pool(name="misc", bufs=2))
        osbp = ctx.enter_context(tc.tile_pool(name="osbp", bufs=6))
        ps_acc = ctx.enter_context(tc.tile_pool(name="ps_acc", bufs=2, space="PSUM"))
        ps_sc = ctx.enter_context(tc.tile_pool(name="ps_sc", bufs=2, space="PSUM"))
        ps_ot = ctx.enter_context(tc.tile_pool(name="ps_ot", bufs=1, space="PSUM"))

        # ---------- input DMA: everything prefetched up front ----------
        # The gpsimd queue is reserved for the latency-critical rope swap
        # DMAs (tiny SBUF->SBUF transfers emitted inside emit_proj) -- bulk
        # loads queued ahead of them starve the rope chain and stall the PE.
        # Big contiguous DMAs (host pre-reordered), ordered by need-time.
        def xchunk(q, tb, c):  # quarter c of x(tb): k-tiles 4c..4c+3
            q.dma_start(XT[tb][:, c * 2048:(c + 1) * 2048],
                        xr[tb * 128:(tb + 1) * 128, c * 2048:(c + 1) * 2048])
        nc.scalar.dma_start(WQs[0][:], wq[0:128, :])
        xchunk(nc.sync, 0, 0)
        xchunk(nc.sync, 0, 1)
        xchunk(nc.scalar, 0, 2)
        xchunk(nc.scalar, 0, 3)
        nc.sync.dma_start(WK[:], wk[:])
        nc.sync.dma_start(WV[:], wv[:])
        nc.scalar.dma_start(C4[:, 0:512], c4[:, 0:512])
        nc.scalar.dma_start(S4[:, 0:512], s4[:, 0:512])
        nc.scalar.dma_start(MSK[:], msk[:])
        nc.sync.dma_start(WQs[1][:], wq[128:256, :])
        nc.scalar.dma_start(WQs[2][:], wq[256:384, :])
        nc.sync.dma_start(WQs[3][:], wq[384:512, :])
        for c in range(4):
            xchunk(nc.sync, 1, c)
        nc.scalar.dma_start(C4[:, 512:2048], c4[:, 512:2048])
        nc.scalar.dma_start(S4[:, 512:2048], s4[:, 512:2048])
        for s in range(SLABS):
            nc.sync.dma_start(WO[s][:], wo[s * 128:(s + 1) * 128, :])
        for c in range(4):
            xchunk(nc.scalar, 2, c)
        for c in range(4):
            xchunk(nc.scalar, 3, c)
        nc.gpsimd.dma_start(V0[:], vones[:])
        nc.gpsimd.dma_start(V1[:], vones[:])

        QTr_by_tb = {}

        def emit_proj(tb):
            t_sl = slice(tb * 512, (tb + 1) * 512)
            xt = XT[tb]
            QTr = [None] * SLABS

            # rope math for group g is emitted only after group g+1's PSUM
            # copy: the copies (which free the accumulator slots) then never
            # queue on the DVE behind swap-DMA-dependent muls.
            lag = []

            def qk_front(s):
                # s < SLABS: q slab s; s == SLABS: k
                ps = ps_acc.tile([128, 512], F32, tag="acc", name="pq")
                for k in range(KTILES):
                    w = WQs[s] if s < SLABS else WK
                    nc.tensor.matmul(ps[:], w[:, k * 128:(k + 1) * 128],
                                     xt[:, k * 512:(k + 1) * 512],
                                     start=(k == 0), stop=(k == KTILES - 1))
                if s < SLABS:
                    dst_t = rot.tile([128, 512], F32R, tag=f"qtr{s}", name=f"qtr{s}")
                    QTr[s] = dst_t
                    dst = dst_t[:]
                else:
                    dst = KT[:, t_sl]
                # single PSUM read (frees the accumulator slot); rest of the
                # rope math is fp16 SBUF-only for the faster DVE tiers
                q_sb = work.tile([128, 512], F32R, tag="qsb", name="qsb")
                nc.vector.tensor_copy(q_sb[:], ps[:])
                # features are laid out [re 0:16 | im 0:16 | re 16:32 | im 16:32]
                # per head so the rope partner is +-16 partitions: one DVE
                # stream_shuffle does the swap (no cross-engine DMA ping-pong)
                q_sw = work.tile([128, 512], F32R, tag="qsw", name="qsw")
                nc.vector.stream_shuffle(q_sw[:], q_sb[:],
                                         list(range(16, 32)) + list(range(16)))
                lag.append((q_sb, q_sw, dst))
                if len(lag) > 1:
                    rope_math(lag.pop(0))

            def rope_math(st):
                q_sb, q_sw, dst = st
                m1 = work.tile([128, 512], F32R, tag="m1", name="m1")
                nc.vector.tensor_mul(m1[:], q_sb[:], C4[:, t_sl])
                m2 = work.tile([128, 512], F32R, tag="m2", name="m2")
                nc.vector.tensor_mul(m2[:], q_sw[:], S4[:, t_sl])
                nc.vector.tensor_add(dst, m1[:], m2[:])

            def rope_flush():
                while lag:
                    rope_math(lag.pop(0))

            def v_group(i):
                sbi = tb * 4 + i
                pv = ps_acc.tile([128, 128], F32, tag="acc", name="pv",
                                 padded_shape=[128, 512])
                for k in range(KTILES):
                    nc.tensor.matmul(pv[:], xt[:, k * 512 + i * 128: k * 512 + (i + 1) * 128],
                                     WV[:, k * 128:(k + 1) * 128],
                                     start=(k == 0), stop=(k == KTILES - 1))
                nc.vector.tensor_copy(V0[:, sbi * 128: sbi * 128 + 64], pv[:, 0:64])
                nc.vector.tensor_copy(V1[:, sbi * 128: sbi * 128 + 64], pv[:, 64:128])

            if tb == 0:
                # attention(0) needs qtr0 + kt + v blocks first; q2/q3 last
                qk_front(0)
                qk_front(SLABS)   # flushes rope(q0)
                qk_front(1)       # flushes rope(k) -> KT ready
                v_group(0)
                rope_flush()      # rope(q1)
                for i in range(1, 4):
                    v_group(i)
                qk_front(2)
                qk_front(3)       # flushes rope(q2)
                rope_flush()
            else:
                # attention(tb) starts from s-block 0: qtr first, own k/v last
                for g in (0, 1, 2, 3, SLABS):
                    qk_front(g)
                rope_flush()
                for i in range(4):
                    v_group(i)
            QTr_by_tb[tb] = QTr

        def emit_attention(qc):
            QTr = QTr_by_tb.pop(qc)
            nblk = (qc + 1) * 4
            OTNr = []
            for s in range(SLABS):
                ot0 = ps_ot.tile([128, 512], F32, tag="ot0", name="ot0")
                ot1 = ps_ot.tile([128, 512], F32, tag="ot1", name="ot1")
                # PV matmuls are emitted one s-block BEHIND the score pair so
                # their scheduler priority is lower than the next score pair:
                # the co-streamed (row-group 0/64) score MMs stay adjacent in
                # the PE queue even when an exp completes mid-pair.
                pend = None

                def flush_pv(sb_):
                    pt_, cl_ = pend
                    nc.tensor.matmul(ot0[:, cl_:512],
                                     V0[:, sb_ * 128: sb_ * 128 + 128],
                                     pt_[:, cl_:512],
                                     start=(sb_ == 0), stop=(sb_ == nblk - 1))
                    nc.tensor.matmul(ot1[:, cl_:512],
                                     V1[:, sb_ * 128: sb_ * 128 + 128],
                                     pt_[:, 512 + cl_:1024],
                                     start=(sb_ == 0), stop=(sb_ == nblk - 1))

                for sb in range(nblk):
                    # columns q < off of this s-block are fully causal-masked:
                    # clip the score matmuls / exp / mask / pv to [cl:512] --
                    # the dead region is never computed NOR read (PSUM
                    # accumulation leaves unwritten columns untouched)
                    off = sb * 128 - qc * 512
                    cl = max(off, 0)
                    sc = ps_sc.tile([128, 1024], F32, tag="sc", name="sc")
                    nc.tensor.matmul(sc[:, cl:512],
                                     KT[0:64, sb * 128:(sb + 1) * 128],
                                     QTr[s][0:64, cl:512], start=True, stop=True)
                    nc.tensor.matmul(sc[:, 512 + cl:1024],
                                     KT[64:128, sb * 128:(sb + 1) * 128],
                                     QTr[s][64:128, cl:512], start=True, stop=True)
                    if sb > 0:
                        flush_pv(sb - 1)
                    pt = ptp.tile([128, 1024], F32R, tag="pt", name="pt")
                    if cl > 0:
                        nc.scalar.activation(
                            pt[:].rearrange("p (h q) -> p h q", h=2)[:, :, cl:],
                            sc[:].rearrange("p (h q) -> p h q", h=2)[:, :, cl:],
                            EXP)
                    else:
                        nc.scalar.activation(pt[:], sc[:], EXP)
                    if off >= 0:  # diagonal s-block: apply causal 0/1 mask
                        # MSK[p, j] = (p + 384 <= j); for q in [cl,512) the
                        # (off + p) <= q mask is cols [384 : 896-off]
                        pt3 = pt[:].rearrange("p (h q) -> p h q", h=2)[:, :, cl:]
                        m3 = (MSK[:, 384:896 - off].unsqueeze(1)
                              .broadcast_to([128, 2, 512 - cl]))
                        nc.vector.tensor_mul(pt3, pt3, m3)
                    pend = (pt, cl)
                flush_pv(nblk - 1)
                otn = rot.tile([128, 512], F32R, tag=f"otnr{s}", name=f"otnr{s}")
                OTNr.append(otn)
                bcs = []
                for h, ot in ((0, ot0), (1, ot1)):
                    dsb = misc.tile([1, 512], F32, tag=f"dsb{h}", name=f"dsb{h}")
                    nc.vector.tensor_copy(dsb[:], ot[64:65, :])
                    rcf = misc.tile([1, 512], F32, tag=f"rcf{h}", name=f"rcf{h}")
                    nc.vector.reciprocal_approx_fast(rcf[:], dsb[:])
                    bc = misc.tile([64, 512], F32, tag=f"bc{h}", name=f"bc{h}")
                    nc.gpsimd.partition_broadcast(bc[:], rcf[:])
                    bcs.append(bc)
                nc.vector.tensor_mul(otn[0:64, :], ot0[0:64, :], bcs[0][:])
                nc.vector.tensor_mul(otn[64:128, :], ot1[0:64, :], bcs[1][:])
            return OTNr

        def emit_wo(tb, OTNr):
            t_sl = slice(tb * 512, (tb + 1) * 512)
            for ob in range(16):
                po = ps_acc.tile([128, 512], F32, tag="acc", name="po")
                for s in range(SLABS):
                    nc.tensor.matmul(po[:], WO[s][:, ob * 128:(ob + 1) * 128],
                                     OTNr[s][:], start=(s == 0), stop=(s == SLABS - 1))
                osb = osbp.tile([128, 512], F32R, tag="osb", name="osb")
                nc.vector.tensor_copy(osb[:], po[:])
                nc.sync.dma_start(outT[ob * 128:(ob + 1) * 128, t_sl], osb[:])

        emit_proj(0)
        for tb in range(TBLK):
            OTNr = emit_attention(tb)
            if tb + 1 < TBLK:
                emit_proj(tb + 1)
            emit_wo(tb, OTNr)

    nc.compile()
    return nc


def _prep_inputs(x, freqs_cos, freqs_sin, wq, wk, wv, wo):
    """Build the 8 per-core input maps (host-side sharding + layout prep)."""
    x = np.asarray(x, dtype=np.float32)
    freqs_cos = np.asarray(freqs_cos, dtype=np.float32)
    freqs_sin = np.asarray(freqs_sin, dtype=np.float32)
    wq = np.asarray(wq, dtype=np.float32)
    wk = np.asarray(wk, dtype=np.float32)
    wv = np.asarray(wv, dtype=np.float32)
    wo = np.asarray(wo, dtype=np.float32)

    # de-interleave permutation within a head: 16-row blocks
    # [re 0:16 | im 0:16 | re 16:32 | im 16:32] so the rope partner sits
    # +-16 partitions away (inside one 32-partition DVE shuffle quadrant)
    ev, od = np.arange(0, HD, 2), np.arange(1, HD, 2)
    deint = np.concatenate([ev[0:16], od[0:16], ev[16:32], od[16:32]])

    # rope tables [128, T] matching that layout: row r (within a head's 64)
    # uses freq j = (r//32)*16 + r%16; sin sign is - on re blocks, + on im
    cosT = freqs_cos.T  # [32, T]
    sinT = freqs_sin.T
    r = np.arange(HD)
    j_of = (r // 32) * 16 + r % 16
    sgn = np.where((r // 16) % 2 == 0, -1.0, 1.0)[:, None]
    c64 = cosT[j_of]
    s64 = sgn * sinT[j_of]
    c4 = np.tile(c64, (2, 1)).astype(np.float32)
    s4 = np.tile(s64, (2, 1)).astype(np.float32)

    vones = np.zeros((128, 16 * 128), dtype=np.float32)
    vones[:, 64::128] = 1.0
    # sliding-window causal mask: msk[p, j] = 1 iff (p + 384) <= j; the
    # diagonal s-block at offset off=128v uses window cols (384-off)..+512
    msk = ((np.arange(128)[:, None] + 384) <= np.arange(896)[None, :]).astype(np.float32)

    def relayout_kp(w, inner):
        # [k*128+p, c] rows -> [p, k*inner+c] (k-tiles side by side)
        return w.reshape(KTILES, 128, inner).transpose(1, 0, 2).reshape(128, KTILES * inner)

    in_maps = []
    for core in range(8):
        b, g = divmod(core, 4)
        # local q head order: slab-major, (s, half) -> global head 8g + s + 4*half
        qheads = [8 * g + s + 4 * h for s in range(SLABS) for h in range(2)]
        kvheads = [2 * g, 2 * g + 1]

        wq_cols = np.concatenate([qh * HD + deint for qh in qheads])
        wk_cols = np.concatenate([kh * HD + deint for kh in kvheads])
        wv_cols = np.concatenate([np.arange(kh * HD, (kh + 1) * HD) for kh in kvheads])
        wo_rows = np.concatenate([np.arange(qh * HD, (qh + 1) * HD) for qh in qheads])

        # xr[tb*128+p, k*512+t] = x^T[k*128+p, tb*512+t]
        xr = (x[b].T.reshape(KTILES, 128, TBLK, 512)
              .transpose(2, 1, 0, 3).reshape(TBLK * 128, KTILES * 512))
        # wqr[s*128+p, k*128+c] = wq_sel[k*128+p, s*128+c]
        wq_sel = wq[:, wq_cols] * (1.0 / np.sqrt(HD))
        wqr = (wq_sel.reshape(KTILES, 128, SLABS, 128)
               .transpose(2, 1, 0, 3).reshape(SLABS * 128, KTILES * 128))

        in_maps.append({
            "xr": _to_mm_dtype(xr),
            "wq": _to_mm_dtype(wqr),
            "wk": _to_mm_dtype(relayout_kp(wk[:, wk_cols], 128)),
            "wv": _to_mm_dtype(relayout_kp(wv[:, wv_cols], 128)),
            "wo": _to_mm_dtype(wo[wo_rows, :]),
            "c4": _to_mm_dtype(c4),
            "s4": _to_mm_dtype(s4),
            "vones": _to_mm_dtype(vones),
            "msk": _to_mm_dtype(msk),
        })
    return in_maps


def kernel(x, freqs_cos, freqs_sin, wq, wk, wv, wo, _trace=False):
    from concourse.bass_utils import run_bass_kernel_spmd

    if "nc" not in _CACHE:
        _CACHE["nc"] = _build()
    nc = _CACHE["nc"]

    in_maps = _prep_inputs(x, freqs_cos, freqs_sin, wq, wk, wv, wo)
    res = run_bass_kernel_spmd(nc, in_maps, core_ids=list(range(8)), trace=_trace)
    _CACHE["last_result"] = res

    out = np.empty((B, T, DIM), dtype=np.float32)
    for b in range(B):
        acc = res.results[4 * b]["outT"].astype(np.float32)
        for g in range(1, 4):
            acc = acc + res.results[4 * b + g]["outT"].astype(np.float32)
        out[b] = acc.T
    return out


# revision 30
# speedup vs baseline: 1.1544x; 1.0132x over previous
"""GQA attention (B=2, T=2048, DIM=2048, NH=32, NKV=8, HD=64) with RoPE, causal,
on 8 TRN2 NeuronCores.

Sharding: data-parallel over B (2) x tensor-parallel over kv-head groups (4).
Core i handles batch i//4 and kv heads {2g, 2g+1} (g = i%4), i.e. q heads
8g..8g+8. wq/wk/wv column-parallel, wo row-parallel; host sums the 4 partial
outputs per batch.

Everything on-device is feature-major ("transposed"): x^T, Q^T, K^T are
[feature, t] so no on-device transposes are needed anywhere:
  QT[d,t] = wq^T x^T;  scoresT[s,q] = (KT slice)^T @ QT;  OT[d,q] = V^T @ PT;
  outT[o,t] = wo^T @ OT.  Host transposes the final [o,t] back to [t,o].

RoPE de-interleave: within each head the 64 features are permuted to
[re 0:16 | im 0:16 | re 16:32 | im 16:32] (host permutes wq/wk columns), so
rot = q*C + swap(q)*S where the +-16-partition swap is a single DVE
stream_shuffle (each 32-partition shuffle quadrant holds a re/im block pair).

Attention inner loop (the key perf structure): for each 128-wide s-block, ONE
PSUM tile sc[128,1024] holds BOTH kv-head halves: cols 0:512 = scores of head
(slab,h0) against KT rows 0:64, cols 512:1024 = head (slab,h1) against KT rows
64:128.  The two K=64 score matmuls land on disjoint PE row groups (0,0) /
(64,0) and are emitted back-to-back with attention MMs at the highest
scheduling priority, so the hardware co-streams them (2x throughput for the
64-deep contraction).  One exp ACT instruction covers both halves (clipped to
the causally-live columns on diagonal s-blocks); causal masking multiplies a
sliding-window 0/1 tile only on the 4 diagonal s-blocks of each q-chunk.  PV
runs as two K=128 matmuls (one per half) accumulating into per-half [128,512]
PSUM tiles; V tiles are padded to 128 cols/s-block (FWL-eligible ldweights)
with an appended ones-column that yields the softmax denominator for free;
1/denom via reciprocal_approx_fast + gpsimd partition_broadcast.

PSUM budget (8 banks): sc 2 bufs x 2 banks + ot_h0/ot_h1 1 buf x 1 bank each
+ proj/wo accumulator 2 bufs x 1 bank = 8.

Emission order per t-block: attention(tb) FIRST (highest priority, keeps the
co-stream pairs adjacent and PV launching right after exp), then proj(tb+1),
then wo(tb) as gap fillers for the ACT-paced exp stream.  All x tiles are
DMA-prefetched at kernel start across the sync/vector/scalar queues so the PE
never waits on input DMA mid-kernel.

All matmul operands are fp16 (KERNEL_MM_DTYPE also allows bf16/f32r); the PE
streams 2-byte operands at full 2.4 GHz rate, fp32 PSUM accumulate.
"""

import numpy as np

B, T, DIM = 2, 2048, 2048
NH, NKV, HD = 32, 8, 64
G = 4            # tensor-parallel groups
QH = NH // G     # 8 local q heads
SLABS = 4
KTILES = DIM // 128
TBLK = T // 512

_CACHE = {}
import os as _os
_MM_DTYPE = _os.environ.get("KERNEL_MM_DTYPE", "fp16")


def _to_mm_dtype(x: np.ndarray) -> np.ndarray:
    if _MM_DTYPE == "bf16":
        import ml_dtypes
        return np.ascontiguousarray(x, dtype=np.float32).astype(ml_dtypes.bfloat16)
    if _MM_DTYPE == "fp16":
        return np.ascontiguousarray(x, dtype=np.float32).astype(np.float16)
    return _round_f32r(x)


def _round_f32r(x: np.ndarray) -> np.ndarray:
    """Round f32 to the float32r grid (11 mantissa bits, round-to-nearest-even)."""
    x = np.ascontiguousarray(x, dtype=np.float32)
    xi = x.view(np.uint32).copy()
    shift = 12  # keep 11 mantissa bits
    lsb = (xi >> shift) & 1
    xi = (xi + ((1 << (shift - 1)) - 1) + lsb) & np.uint32(~((1 << shift) - 1) & 0xFFFFFFFF)
    return xi.view(np.float32)


def _build():
    import concourse.bass as bass
    import concourse.mybir as mybir
    import concourse.tile as tile
    from concourse import bacc

    F32 = mybir.dt.float32
    F32R = {"bf16": mybir.dt.bfloat16, "fp16": mybir.dt.float16,
            "f32r": mybir.dt.float32r}[_MM_DTYPE]
    EXP = mybir.ActivationFunctionType.Exp

    nc = bacc.Bacc("TRN2", target_bir_lowering=False, debug=False, num_devices=8)

    # host pre-reorders weights/x so each SBUF tile loads with ONE big DMA
    # (per-DMA overhead ~0.5-1us and same-tile writes serialize, so many
    # small loads gate the kernel start):
    #   xr[tb*128+p, k*512+t] = x^T[k*128+p, tb*512+t]
    #   wqr[s*128+p, k*128+c] = wq_cols[k*128+p, s*128+c] / sqrt(hd)
    #   wkr[p, k*128+c] = wk_cols[k*128+p, c]   (wvr likewise)
    xr = nc.dram_tensor("xr", [4 * 128, KTILES * 512], F32R, kind="ExternalInput").ap()
    wq = nc.dram_tensor("wq", [SLABS * 128, KTILES * 128], F32R, kind="ExternalInput").ap()
    wk = nc.dram_tensor("wk", [128, KTILES * 128], F32R, kind="ExternalInput").ap()
    wv = nc.dram_tensor("wv", [128, KTILES * 128], F32R, kind="ExternalInput").ap()
    wo = nc.dram_tensor("wo", [QH * HD, DIM], F32R, kind="ExternalInput").ap()
    c4 = nc.dram_tensor("c4", [128, T], F32R, kind="ExternalInput").ap()
    s4 = nc.dram_tensor("s4", [128, T], F32R, kind="ExternalInput").ap()
    vones = nc.dram_tensor("vones", [128, 16 * 128], F32R, kind="ExternalInput").ap()
    msk = nc.dram_tensor("msk", [128, 896], F32R, kind="ExternalInput").ap()
    outT = nc.dram_tensor("outT", [DIM, T], F32R, kind="ExternalOutput").ap()

    from contextlib import ExitStack

    with tile.TileContext(nc) as tc, ExitStack() as ctx:
        # ---------- persistent tiles ----------
        pers = ctx.enter_context(tc.tile_pool(name="pers", bufs=1))
        KT = pers.tile([128, T], F32R, tag="kt", name="kt")
        # v tiles padded to 128 cols per s-block (cols 65:128 zero) so the
        # pv ldweights is a full 128-col load (fast-weight-load eligible)
        V0 = pers.tile([128, 16 * 128], F32R, tag="v0", name="v0")
        V1 = pers.tile([128, 16 * 128], F32R, tag="v1", name="v1")
        MSK = pers.tile([128, 896], F32R, tag="msk", name="msk_sb")
        C4 = pers.tile([128, T], F32R, tag="c4", name="c4_sb")
        S4 = pers.tile([128, T], F32R, tag="s4", name="s4_sb")
        WQs = [pers.tile([128, KTILES * 128], F32R, tag=f"wq{s}", name=f"wq{s}")
               for s in range(SLABS)]
        WK = pers.tile([128, KTILES * 128], F32R, tag="wk", name="wk_sb")
        WV = pers.tile([128, KTILES * 128], F32R, tag="wv", name="wv_sb")
        WO = [pers.tile([128, T], F32R, tag=f"wo{s}", name=f"wo{s}") for s in range(SLABS)]
        # all x resident for the whole kernel: XT[tb] = [128, k*512+t]
        XT = [pers.tile([128, KTILES * 512], F32R, tag=f"x{tb}", name=f"x{tb}")
              for tb in range(TBLK)]

        rot = ctx.enter_context(tc.tile_pool(name="rot", bufs=2))
        work = ctx.enter_context(tc.tile_pool(name="work", bufs=2))
        ptp = ctx.enter_context(tc.tile_pool(name="ptp", bufs=6))
        misc = ctx.enter_context(tc.tile_pool(name="misc", bufs=2))
        osbp = ctx.enter_context(tc.tile_pool(name="osbp", bufs=6))
        ps_acc = ctx.enter_context(tc.tile_pool(name="ps_acc", bufs=2, space="PSUM"))
        ps_sc = ctx.enter_context(tc.tile_pool(name="ps_sc", bufs=2, space="PSUM"))
        ps_ot = ctx.enter_context(tc.tile_pool(name="ps_ot", bufs=1, space="PSUM"))

        # ---------- input DMA: everything prefetched up front ----------
        # The gpsimd queue is reserved for the latency-critical rope swap
        # DMAs (tiny SBUF->SBUF transfers emitted inside emit_proj) -- bulk
        # loads queued ahead of them starve the rope chain and stall the PE.
        # Big contiguous DMAs (host pre-reordered), ordered by need-time.
        def xchunk(q, tb, c):  # quarter c of x(tb): k-tiles 4c..4c+3
            q.dma_start(XT[tb][:, c * 2048:(c + 1) * 2048],
                        xr[tb * 128:(tb + 1) * 128, c * 2048:(c + 1) * 2048])
        # scalar engine also runs the exps: give it ONLY the start-critical
        # loads (its queue head-blocks on DMA-ring slots); x2/x3 descriptors
        # are emitted lazily inside later proj phases (2 at a time, below
        # ring depth).  Everything else rides the sync queue.
        nc.scalar.dma_start(WQs[0][:], wq[0:128, :])
        xchunk(nc.sync, 0, 0)
        xchunk(nc.sync, 0, 1)
        xchunk(nc.scalar, 0, 2)
        xchunk(nc.scalar, 0, 3)
        nc.scalar.dma_start(C4[:, 0:512], c4[:, 0:512])
        nc.scalar.dma_start(S4[:, 0:512], s4[:, 0:512])
        nc.scalar.dma_start(MSK[:], msk[:])
        nc.sync.dma_start(WK[:], wk[:])
        nc.sync.dma_start(WV[:], wv[:])
        nc.sync.dma_start(WQs[1][:], wq[128:256, :])
        nc.sync.dma_start(WQs[2][:], wq[256:384, :])
        nc.sync.dma_start(WQs[3][:], wq[384:512, :])
        for c in range(4):
            xchunk(nc.sync, 1, c)
        for s in range(SLABS):
            nc.sync.dma_start(WO[s][:], wo[s * 128:(s + 1) * 128, :])
        nc.gpsimd.dma_start(V0[:], vones[:])
        nc.gpsimd.dma_start(V1[:], vones[:])
        nc.gpsimd.dma_start(C4[:, 512:2048], c4[:, 512:2048])
        nc.gpsimd.dma_start(S4[:, 512:2048], s4[:, 512:2048])

        QTr_by_tb = {}

        def emit_proj(tb):
            t_sl = slice(tb * 512, (tb + 1) * 512)
            xt = XT[tb]
            QTr = [None] * SLABS

            # rope math for group g is emitted only after group g+1's PSUM
            # copy: the copies (which free the accumulator slots) then never
            # queue on the DVE behind swap-DMA-dependent muls.
            lag = []

            def qk_front(s):
                # s < SLABS: q slab s; s == SLABS: k
                ps = ps_acc.tile([128, 512], F32, tag="acc", name="pq")
                for k in range(KTILES):
                    w = WQs[s] if s < SLABS else WK
                    nc.tensor.matmul(ps[:], w[:, k * 128:(k + 1) * 128],
                                     xt[:, k * 512:(k + 1) * 512],
                                     start=(k == 0), stop=(k == KTILES - 1))
                if s < SLABS:
                    dst_t = rot.tile([128, 512], F32R, tag=f"qtr{s}", name=f"qtr{s}")
                    QTr[s] = dst_t
                    dst = dst_t[:]
                else:
                    dst = KT[:, t_sl]
                # single PSUM read (frees the accumulator slot); rest of the
                # rope math is fp16 SBUF-only for the faster DVE tiers
                q_sb = work.tile([128, 512], F32R, tag="qsb", name="qsb")
                nc.vector.tensor_copy(q_sb[:], ps[:])
                # features are laid out [re 0:16 | im 0:16 | re 16:32 | im 16:32]
                # per head so the rope partner is +-16 partitions: one DVE
                # stream_shuffle does the swap (no cross-engine DMA ping-pong)
                q_sw = work.tile([128, 512], F32R, tag="qsw", name="qsw")
                nc.vector.stream_shuffle(q_sw[:], q_sb[:],
                                         list(range(16, 32)) + list(range(16)))
                lag.append((q_sb, q_sw, dst))
                if len(lag) > 1:
                    rope_math(lag.pop(0))

            def rope_math(st):
                q_sb, q_sw, dst = st
                m1 = work.tile([128, 512], F32R, tag="m1", name="m1")
                nc.vector.tensor_mul(m1[:], q_sb[:], C4[:, t_sl])
                m2 = work.tile([128, 512], F32R, tag="m2", name="m2")
                nc.vector.tensor_mul(m2[:], q_sw[:], S4[:, t_sl])
                nc.vector.tensor_add(dst, m1[:], m2[:])

            def rope_flush():
                while lag:
                    rope_math(lag.pop(0))

            def v_group(i):
                sbi = tb * 4 + i
                pv = ps_acc.tile([128, 128], F32, tag="acc", name="pv",
                                 padded_shape=[128, 512])
                for k in range(KTILES):
                    nc.tensor.matmul(pv[:], xt[:, k * 512 + i * 128: k * 512 + (i + 1) * 128],
                                     WV[:, k * 128:(k + 1) * 128],
                                     start=(k == 0), stop=(k == KTILES - 1))
                nc.vector.tensor_copy(V0[:, sbi * 128: sbi * 128 + 64], pv[:, 0:64])
                nc.vector.tensor_copy(V1[:, sbi * 128: sbi * 128 + 64], pv[:, 64:128])

            if tb == 0:
                # attention(0) needs qtr0 + kt + v blocks first; q2/q3 last
                qk_front(0)
                qk_front(SLABS)   # flushes rope(q0)
                qk_front(1)       # flushes rope(k) -> KT ready
                v_group(0)
                rope_flush()      # rope(q1)
                for i in range(1, 4):
                    v_group(i)
                qk_front(2)
                qk_front(3)       # flushes rope(q2)
                rope_flush()
            else:
                # attention(tb) starts from s-block 0: qtr first, own k/v last
                for g in (0, 1, 2, 3, SLABS):
                    qk_front(g)
                rope_flush()
                for i in range(4):
                    v_group(i)
            QTr_by_tb[tb] = QTr

        def emit_attention(qc):
            QTr = QTr_by_tb.pop(qc)
            nblk = (qc + 1) * 4
            OTNr = []
            for s in range(SLABS):
                ot0 = ps_ot.tile([128, 512], F32, tag="ot0", name="ot0")
                ot1 = ps_ot.tile([128, 512], F32, tag="ot1", name="ot1")
                # PV matmuls are emitted one s-block BEHIND the score pair so
                # their scheduler priority is lower than the next score pair:
                # the co-streamed (row-group 0/64) score MMs stay adjacent in
                # the PE queue even when an exp completes mid-pair.
                pend = None

                def flush_pv(sb_):
                    pt_, cl_ = pend
                    nc.tensor.matmul(ot0[:, cl_:512],
                                     V0[:, sb_ * 128: sb_ * 128 + 128],
                                     pt_[:, cl_:512],
                                     start=(sb_ == 0), stop=(sb_ == nblk - 1))
                    nc.tensor.matmul(ot1[:, cl_:512],
                                     V1[:, sb_ * 128: sb_ * 128 + 128],
                                     pt_[:, 512 + cl_:1024],
                                     start=(sb_ == 0), stop=(sb_ == nblk - 1))

                for sb in range(nblk):
                    # columns q < off of this s-block are fully causal-masked:
                    # clip the score matmuls / exp / mask / pv to [cl:512] --
                    # the dead region is never computed NOR read (PSUM
                    # accumulation leaves unwritten columns untouched)
                    off = sb * 128 - qc * 512
                    cl = max(off, 0)
                    sc = ps_sc.tile([128, 1024], F32, tag="sc", name="sc")
                    nc.tensor.matmul(sc[:, cl:512],
                                     KT[0:64, sb * 128:(sb + 1) * 128],
                                     QTr[s][0:64, cl:512], start=True, stop=True)
                    nc.tensor.matmul(sc[:, 512 + cl:1024],
                                     KT[64:128, sb * 128:(sb + 1) * 128],
                                     QTr[s][64:128, cl:512], start=True, stop=True)
                    if sb > 0:
                        flush_pv(sb - 1)
                    pt = ptp.tile([128, 1024], F32R, tag="pt", name="pt")
                    if cl > 0:
                        nc.scalar.activation(
                            pt[:].rearrange("p (h q) -> p h q", h=2)[:, :, cl:],
                            sc[:].rearrange("p (h q) -> p h q", h=2)[:, :, cl:],
                            EXP)
                    else:
                        nc.scalar.activation(pt[:], sc[:], EXP)
                    if off >= 0:  # diagonal s-block: apply causal 0/1 mask
                        # MSK[p, j] = (p + 384 <= j); for q in [cl,512) the
                        # (off + p) <= q mask is cols [384 : 896-off]
                        pt3 = pt[:].rearrange("p (h q) -> p h q", h=2)[:, :, cl:]
                        m3 = (MSK[:, 384:896 - off].unsqueeze(1)
                              .broadcast_to([128, 2, 512 - cl]))
                        nc.vector.tensor_mul(pt3, pt3, m3)
                    pend = (pt, cl)
                flush_pv(nblk - 1)
                otn = rot.tile([128, 512], F32R, tag=f"otnr{s}", name=f"otnr{s}")
                OTNr.append(otn)
                bcs = []
                for h, ot in ((0, ot0), (1, ot1)):
                    dsb = misc.tile([1, 512], F32, tag=f"dsb{h}", name=f"dsb{h}")
                    nc.vector.tensor_copy(dsb[:], ot[64:65, :])
                    rcf = misc.tile([1, 512], F32, tag=f"rcf{h}", name=f"rcf{h}")
                    nc.vector.reciprocal_approx_fast(rcf[:], dsb[:])
                    bc = misc.tile([64, 512], F32, tag=f"bc{h}", name=f"bc{h}")
                    nc.gpsimd.partition_broadcast(bc[:], rcf[:])
                    bcs.append(bc)
                nc.vector.tensor_mul(otn[0:64, :], ot0[0:64, :], bcs[0][:])
                nc.vector.tensor_mul(otn[64:128, :], ot1[0:64, :], bcs[1][:])
            return OTNr

        def emit_wo(tb, OTNr):
            t_sl = slice(tb * 512, (tb + 1) * 512)
            for ob in range(16):
                po = ps_acc.tile([128, 512], F32, tag="acc", name="po")
                for s in range(SLABS):
                    nc.tensor.matmul(po[:], WO[s][:, ob * 128:(ob + 1) * 128],
                                     OTNr[s][:], start=(s == 0), stop=(s == SLABS - 1))
                osb = osbp.tile([128, 512], F32R, tag="osb", name="osb")
                nc.vector.tensor_copy(osb[:], po[:])
                nc.sync.dma_start(outT[ob * 128:(ob + 1) * 128, t_sl], osb[:])

        emit_proj(0)
        for tb in range(TBLK):
            OTNr = emit_attention(tb)
            if tb + 2 < TBLK:
                # x(tb+2) load: two half-tile descriptors, emitted only now so
                # the scalar queue never ring-blocks ahead of attention exps
                nc.scalar.dma_start(XT[tb + 2][:, 0:4096],
                                    xr[(tb + 2) * 128:(tb + 3) * 128, 0:4096])
                nc.scalar.dma_start(XT[tb + 2][:, 4096:8192],
                                    xr[(tb + 2) * 128:(tb + 3) * 128, 4096:8192])
            if tb + 1 < TBLK:
                emit_proj(tb + 1)
            emit_wo(tb, OTNr)

    nc.compile()
    return nc


def _prep_inputs(x, freqs_cos, freqs_sin, wq, wk, wv, wo):
    """Build the 8 per-core input maps (host-side sharding + layout prep)."""
    x = np.asarray(x, dtype=np.float32)
    freqs_cos = np.asarray(freqs_cos, dtype=np.float32)
    freqs_sin = np.asarray(freqs_sin, dtype=np.float32)
    wq = np.asarray(wq, dtype=np.float32)
    wk = np.asarray(wk, dtype=np.float32)
    wv = np.asarray(wv, dtype=np.float32)
    wo = np.asarray(wo, dtype=np.float32)

    # de-interleave permutation within a head: 16-row blocks
    # [re 0:16 | im 0:16 | re 16:32 | im 16:32] so the rope partner sits
    # +-16 partitions away (inside one 32-partition DVE shuffle quadrant)
    ev, od = np.arange(0, HD, 2), np.arange(1, HD, 2)
    deint = np.concatenate([ev[0:16], od[0:16], ev[16:32], od[16:32]])

    # rope tables [128, T] matching that layout: row r (within a head's 64)
    # uses freq j = (r//32)*16 + r%16; sin sign is - on re blocks, + on im
    cosT = freqs_cos.T  # [32, T]
    sinT = freqs_sin.T
    r = np.arange(HD)
    j_of = (r // 32) * 16 + r % 16
    sgn = np.where((r // 16) % 2 == 0, -1.0, 1.0)[:, None]
    c64 = cosT[j_of]
    s64 = sgn * sinT[j_of]
    c4 = np.tile(c64, (2, 1)).astype(np.float32)
    s4 = np.tile(s64, (2, 1)).astype(np.float32)

    vones = np.zeros((128, 16 * 128), dtype=np.float32)
    vones[:, 64::128] = 1.0
    # sliding-window causal mask: msk[p, j] = 1 iff (p + 384) <= j; the
    # diagonal s-block at offset off=128v uses window cols (384-off)..+512
    msk = ((np.arange(128)[:, None] + 384) <= np.arange(896)[None, :]).astype(np.float32)

    def relayout_kp(w, inner):
        # [k*128+p, c] rows -> [p, k*inner+c] (k-tiles side by side)
        return w.reshape(KTILES, 128, inner).transpose(1, 0, 2).reshape(128, KTILES * inner)

    in_maps = []
    for core in range(8):
        b, g = divmod(core, 4)
        # local q head order: slab-major, (s, half) -> global head 8g + s + 4*half
        qheads = [8 * g + s + 4 * h for s in range(SLABS) for h in range(2)]
        kvheads = [2 * g, 2 * g + 1]

        wq_cols = np.concatenate([qh * HD + deint for qh in qheads])
        wk_cols = np.concatenate([kh * HD + deint for kh in kvheads])
        wv_cols = np.concatenate([np.arange(kh * HD, (kh + 1) * HD) for kh in kvheads])
        wo_rows = np.concatenate([np.arange(qh * HD, (qh + 1) * HD) for qh in qheads])

        # xr[tb*128+p, k*512+t] = x^T[k*128+p, tb*512+t]
        xr = (x[b].T.reshape(KTILES, 128, TBLK, 512)
              .transpose(2, 1, 0, 3).reshape(TBLK * 128, KTILES * 512))
        # wqr[s*128+p, k*128+c] = wq_sel[k*128+p, s*128+c]
        wq_sel = wq[:, wq_cols] * (1.0 / np.sqrt(HD))
        wqr = (wq_sel.reshape(KTILES, 128, SLABS, 128)
               .transpose(2, 1, 0, 3).reshape(SLABS * 128, KTILES * 128))

        in_maps.append({
            "xr": _to_mm_dtype(xr),
            "wq": _to_mm_dtype(wqr),
            "wk": _to_mm_dtype(relayout_kp(wk[:, wk_cols], 128)),
            "wv": _to_mm_dtype(relayout_kp(wv[:, wv_cols], 128)),
            "wo": _to_mm_dtype(wo[wo_rows, :]),
            "c4": _to_mm_dtype(c4),
            "s4": _to_mm_dtype(s4),
            "vones": _to_mm_dtype(vones),
            "msk": _to_mm_dtype(msk),
        })
    return in_maps


def kernel(x, freqs_cos, freqs_sin, wq, wk, wv, wo, _trace=False):
    from concourse.bass_utils import run_bass_kernel_spmd

    if "nc" not in _CACHE:
        _CACHE["nc"] = _build()
    nc = _CACHE["nc"]

    in_maps = _prep_inputs(x, freqs_cos, freqs_sin, wq, wk, wv, wo)
    res = run_bass_kernel_spmd(nc, in_maps, core_ids=list(range(8)), trace=_trace)
    _CACHE["last_result"] = res

    out = np.empty((B, T, DIM), dtype=np.float32)
    for b in range(B):
        acc = res.results[4 * b]["outT"].astype(np.float32)
        for g in range(1, 4):
            acc = acc + res.results[4 * b + g]["outT"].astype(np.float32)
        out[b] = acc.T
    return out
